# revision 1
# baseline (speedup 1.0000x reference)
"""Multi-head self-attention (B=2, S=2048, D=1024, H=16) on 8 Trainium2 NeuronCores.

Sharding: batch x head-group. Core c = b*4 + g handles batch b and heads 4g..4g+3
(Megatron-style TP: Wq/Wk/Wv column-sharded, Wo row-sharded; partial outputs
summed on the host).

Per-core kernel layout ("T-layout": sequence on the free dim everywhere):
  inputs (host-prepared):  xt [1024, 2048] = x[b].T;  wq/wk/wv [1024, 256]
  (scale-folded, transposed);  wo [256, 1024] (scale-folded, transposed)
  QT/KT = (w.T @ xt) [256, 2048]        d' on partitions, heads pair-stacked
  V     = (xt.T @ wv) [2048, 260]       natural layout + ones column per head
  scoresT[k, q] = KT_h-slices.T @ QT_h  per head, k on partitions
  expT = exp(scoresT / 8)               (no max subtraction: |scores| <~ 2)
  ctxT_aug[d+1, q] = [V_h | 1].T @ expT accumulated over k-chunks; row 64 = denom
  ctxT = ctxT_aug[0:64] * (1/denom)     denominator broadcast via gpsimd
  outT_partial = wo.T @ ctxT [1024, 2048]
Host: out[b] = sum_g outT[b, g].T

Every matmul uses K<=64 contraction (row-tiled 64x128 PE mode, tiles T0/T8
run concurrently) so the PE never switches tiling modes.
"""
import sys

sys.path.insert(0, "/opt/trn_rl_repo")

import numpy as np

import concourse.bass as bass
import concourse.tile as tile
from concourse import bacc, mybir
from concourse.bass_utils import run_bass_kernel_spmd

F32 = mybir.dt.float32
MM_DT = mybir.dt.float32r  # 1 cycle/row at N>=256 (fp32 is 4); fp32 storage

S = 2048          # sequence length per batch
D = 1024          # embedding dim
HG = 4            # heads per core
HD = 64           # head dim
GC = HG * HD      # group cols = 256
P = 128
NQ = 4            # q chunks of 512
QW = 512          # q chunk width
NKC = 16          # key-position chunks of 128
KO = 8            # contraction chunks of 128 over D
VW = HD + 1       # V columns per head incl. ones column

_NC_CACHE = {}
DEBUG_DUMPS = False


def _build():
    if "nc" in _NC_CACHE:
        return _NC_CACHE["nc"]
    nc = bacc.Bacc(trn_type="TRN2", target_bir_lowering=False, debug=False)
    xt_d = nc.dram_tensor("xt", [D, S], MM_DT, kind="ExternalInput")
    wq_d = nc.dram_tensor("wq", [D, GC], MM_DT, kind="ExternalInput")
    wk_d = nc.dram_tensor("wk", [D, GC], MM_DT, kind="ExternalInput")
    wv_d = nc.dram_tensor("wv", [D, GC], MM_DT, kind="ExternalInput")
    wo_d = nc.dram_tensor("wo", [GC, D], MM_DT, kind="ExternalInput")
    out_d = nc.dram_tensor("out_t", [D, S], F32, kind="ExternalOutput")
    dbg = None
    if DEBUG_DUMPS:
        dbg = {
            "dbg_qt": nc.dram_tensor("dbg_qt", [P, 2, S], MM_DT, kind="ExternalOutput"),
            "dbg_kt": nc.dram_tensor("dbg_kt", [P, 2, S], MM_DT, kind="ExternalOutput"),
            "dbg_va": nc.dram_tensor("dbg_va", [P, NKC, HG * VW], MM_DT,
                                     kind="ExternalOutput"),
            "dbg_ct": nc.dram_tensor("dbg_ct", [P, 2, S], MM_DT, kind="ExternalOutput"),
            "dbg_ex": nc.dram_tensor("dbg_ex", [P, 4, QW], MM_DT, kind="ExternalOutput"),
            "dbg_sc": nc.dram_tensor("dbg_sc", [P, 4, QW], F32, kind="ExternalOutput"),
        }

    scr_d = nc.dram_tensor("nrm_scratch", [2, NQ, 2, QW], F32)
    with tile.TileContext(nc) as tc:
        _emit(nc, tc, xt_d, wq_d, wk_d, wv_d, wo_d, out_d, scr_d, dbg)
    nc.compile()
    _NC_CACHE["nc"] = nc
    return nc


def _emit(nc, tc, xt_d, wq_d, wk_d, wv_d, wo_d, out_d, scr_d, dbg=None):
    with tc.tile_pool(name="big", bufs=1) as big:
        # ---- persistent SBUF tensors (~96KB/partition) ----
        wo_sb = big.tile([P, 2, D], MM_DT)        # [d'(128) x chunk x e]
        qt = big.tile([P, 2, S], MM_DT)           # QT: head h at parts (h%2)*64, chunk h//2
        kt = big.tile([P, 2, S], MM_DT)
        va = big.tile([P, NKC, HG * VW], MM_DT)   # V natural + ones col per head
        ct = big.tile([P, 2, S], MM_DT)           # ctxT, same head layout as qt

        nc.sync.dma_start(wo_sb[:], wo_d.rearrange("(c p) e -> p c e", p=P))

        # ones columns of V_aug (col HD of each VW-wide head block)
        va_h = va[:].rearrange("p s (h c) -> p s h c", c=VW)
        for h in range(HG):
            # fp32 1.0 bit pattern; walrus memset rejects float32r directly
            nc.vector.memset(
                va_h[:, :, h, HD:HD + 1].bitcast(mybir.dt.uint32), 0x3F800000)

        def mm_pair(pa, pb, lhsT, rhs, start, stop):
            """Row-tiled K=64 pair: T0 (parts 0-63) -> pa, T8 (parts 64-127) -> pb."""
            nc.tensor.matmul(pa, lhsT[0:64], rhs[0:64], start=start, stop=stop)
            nc.tensor.matmul(pb, lhsT[64:128], rhs[64:128], start=start, stop=stop)

        # ================= phase 1: projections =================
        with tc.tile_pool(name="xw", bufs=1) as xw, \
             tc.tile_pool(name="evac", bufs=3) as evac, \
             tc.tile_pool(name="ps_proj", bufs=4, space="PSUM") as ps_proj:
            xs = xw.tile([P, KO, S], MM_DT)       # x.T, [d_in(128) x ko x s]
            wq = xw.tile([P, KO, GC], MM_DT)
            wk = xw.tile([P, KO, GC], MM_DT)
            wv = xw.tile([P, KO, GC], MM_DT)
            for ko in range(KO):
                nc.sync.dma_start(xs[:, ko, :], xt_d[ko * P:(ko + 1) * P, :])
            nc.sync.dma_start(wq[:], wq_d.rearrange("(ko p) m -> p ko m", p=P))
            nc.sync.dma_start(wk[:], wk_d.rearrange("(ko p) m -> p ko m", p=P))
            nc.sync.dma_start(wv[:], wv_d.rearrange("(ko p) m -> p ko m", p=P))

            # QT/KT: transposed out [d' x s]
            for w_sb, dst in ((wq, qt), (wk, kt)):
                for m in range(2):          # d' chunk = head pair
                    for n in range(NQ):
                        pa = ps_proj.tile([P, QW], F32, tag="pp")
                        pb = ps_proj.tile([P, QW], F32, tag="pp")
                        for ko in range(KO):
                            mm_pair(pa[:], pb[:],
                                    w_sb[:, ko, m * P:(m + 1) * P],
                                    xs[:, ko, n * QW:(n + 1) * QW],
                                    start=(ko == 0), stop=(ko == KO - 1))
                        t = evac.tile([P, QW], F32, tag="ev")
                        nc.vector.tensor_copy(t[:], pb[:])
                        nc.vector.tensor_tensor(
                            dst[:, m, n * QW:(n + 1) * QW],
                            pa[:], t[:], mybir.AluOpType.add)

            # V natural: [s(128) x 256] per s-chunk
            for sc in range(NKC):
                pa = ps_proj.tile([P, QW], F32, tag="pp")
                pb = ps_proj.tile([P, QW], F32, tag="pp")
                for ko in range(KO):
                    mm_pair(pa[:, :GC], pb[:, :GC],
                            xs[:, ko, sc * P:(sc + 1) * P],
                            wv[:, ko, :],
                            start=(ko == 0), stop=(ko == KO - 1))
                tv = evac.tile([P, QW], F32, tag="ev")
                nc.vector.tensor_copy(tv[:, :GC], pb[:, :GC])
                nc.vector.tensor_tensor(
                    va_h[:, sc, :, 0:HD],
                    pa[:, :GC].rearrange("p (h c) -> p h c", c=HD),
                    tv[:, :GC].rearrange("p (h c) -> p h c", c=HD),
                    mybir.AluOpType.add)

        if dbg is not None:
            nc.sync.dma_start(dbg["dbg_qt"][:], qt[:])
            nc.sync.dma_start(dbg["dbg_kt"][:], kt[:])
            nc.sync.dma_start(dbg["dbg_va"][:], va[:])

        # ================= phase 2: attention =================
        with tc.tile_pool(name="expp", bufs=2) as expp, \
             tc.tile_pool(name="norm", bufs=2) as norm, \
             tc.tile_pool(name="ps_sc", bufs=1, space="PSUM") as ps_sc, \
             tc.tile_pool(name="ps_ctx", bufs=1, space="PSUM") as ps_ctx:
            for hp in range(2):         # head pair (even = parts 0-63, odd = 64-127)
                for n in range(NQ):
                    # ctx accumulators: [even/odd head] x [k-low/k-high half]
                    cps = [[ps_ctx.tile([P, QW], F32, tag=f"pc{e}{l}",
                                        name=f"pc{e}{l}_{hp}_{n}")
                            for l in range(2)] for e in range(2)]
                    for kb in range(NKC // 2):
                        sp = ps_sc.tile([P, 4, QW], F32, tag="psc")   # 4 banks
                        ex = expp.tile([P, 4, QW], MM_DT, tag="pex")
                        for j in range(4):
                            kc = kb * 2 + j // 2
                            lo = (j % 2) * 64
                            nc.tensor.matmul(
                                sp[:, j, :],
                                kt[lo:lo + 64, hp, kc * P:(kc + 1) * P],
                                qt[lo:lo + 64, hp, n * QW:(n + 1) * QW],
                                start=True, stop=True)
                        nc.scalar.activation(
                            ex[:].rearrange("p a b -> p (a b)"),
                            sp[:].rearrange("p a b -> p (a b)"),
                            mybir.ActivationFunctionType.Exp,
                            scale=0.125)
                        if dbg is not None and hp == 0 and n == 0 and kb == 0:
                            nc.sync.dma_start(dbg["dbg_ex"][:], ex[:])
                            spc = norm.tile([P, 4, QW], F32, tag="spdump")
                            nc.vector.tensor_copy(spc[:], sp[:])
                            nc.sync.dma_start(dbg["dbg_sc"][:], spc[:])
                        for j in range(4):
                            kc = kb * 2 + j // 2
                            e = j % 2
                            h = 2 * hp + e
                            for l in range(2):   # k-low / k-high 64-halves
                                nc.tensor.matmul(
                                    cps[e][l][0:VW, :],
                                    va[l * 64:(l + 1) * 64, kc, h * VW:(h + 1) * VW],
                                    ex[l * 64:(l + 1) * 64, j, :],
                                    start=(kb == 0 and j < 2),
                                    stop=(kb == NKC // 2 - 1 and j >= 2))
                    # normalize: ctxT = (A+B)[0:64] / (A+B)[64]
                    for e in range(2):
                        sm = norm.tile([P, QW], F32, tag="nsum")
                        bc = norm.tile([P, QW], F32, tag="nbc")
                        nc.vector.tensor_copy(sm[0:VW, :], cps[e][1][0:VW, :])
                        nc.vector.tensor_tensor(sm[0:VW, :], cps[e][0][0:VW, :],
                                                sm[0:VW, :], mybir.AluOpType.add)
                        nc.vector.reciprocal(sm[HD:VW, :], sm[HD:VW, :])
                        # partition-broadcast 1/denom via DRAM bounce
                        sl = scr_d[hp, n, e]
                        nc.sync.dma_start(sl.unsqueeze(0), sm[HD:VW, :])
                        bc_src = bass.AP(tensor=sl.tensor, offset=sl.offset,
                                         ap=[[0, 64]] + list(sl.ap))
                        nc.sync.dma_start(bc[0:64, :], bc_src)
                        nc.vector.tensor_tensor(
                            ct[e * 64:e * 64 + 64, hp, n * QW:(n + 1) * QW],
                            sm[0:HD, :], bc[0:64, :], mybir.AluOpType.mult)

        if dbg is not None:
            nc.sync.dma_start(dbg["dbg_ct"][:], ct[:])

        # ================= phase 3: output projection =================
        with tc.tile_pool(name="outp", bufs=3) as outp, \
             tc.tile_pool(name="ps_o", bufs=4, space="PSUM") as ps_o:
            for m in range(KO):         # e chunks of 128
                for n in range(NQ):
                    pa = ps_o.tile([P, QW], F32, tag="po")
                    pb = ps_o.tile([P, QW], F32, tag="po")
                    for c in range(2):
                        mm_pair(pa[:], pb[:],
                                wo_sb[:, c, m * P:(m + 1) * P],
                                ct[:, c, n * QW:(n + 1) * QW],
                                start=(c == 0), stop=(c == 1))
                    ot = outp.tile([P, QW], F32, tag="ot")
                    nc.vector.tensor_copy(ot[:], pb[:])
                    nc.vector.tensor_tensor(ot[:], pa[:], ot[:],
                                            mybir.AluOpType.add)
                    nc.sync.dma_start(
                        out_d[m * P:(m + 1) * P, n * QW:(n + 1) * QW], ot[:])


def _in_maps(x, wq_f, wk_f, wv_f, wo_f):
    maps = []
    for core in range(8):
        b, g = core // 4, core % 4
        cols = slice(g * GC, (g + 1) * GC)
        maps.append({
            "xt": np.ascontiguousarray(x[b].T),
            "wq": np.ascontiguousarray(wq_f[:, cols]),
            "wk": np.ascontiguousarray(wk_f[:, cols]),
            "wv": np.ascontiguousarray(wv_f[:, cols]),
            "wo": np.ascontiguousarray(wo_f[cols, :]),
        })
    return maps


def run_traced(x, Wq, Wk, Wv, Wo, q_scale, k_scale, v_scale, o_scale):
    """Like kernel() but with NTFF tracing; returns (out, exec_time_ns, trace_path)."""
    x = np.asarray(x, dtype=np.float32)
    wq_f = (np.asarray(Wq).T * np.asarray(q_scale).reshape(1, -1)).astype(np.float32)
    wk_f = (np.asarray(Wk).T * np.asarray(k_scale).reshape(1, -1)).astype(np.float32)
    wv_f = (np.asarray(Wv).T * np.asarray(v_scale).reshape(1, -1)).astype(np.float32)
    wo_f = (np.asarray(Wo).T * np.asarray(o_scale).reshape(1, -1)).astype(np.float32)
    nc = _build()
    res = run_bass_kernel_spmd(nc, _in_maps(x, wq_f, wk_f, wv_f, wo_f),
                               core_ids=list(range(8)), trace=True)
    out = np.zeros((x.shape[0], S, D), dtype=np.float32)
    for core in range(8):
        out[core // 4] += res.results[core]["out_t"].T
    trace_path = None
    if res.instructions_and_trace is not None:
        trace_path = res.instructions_and_trace[1]
    return out, res.exec_time_ns, trace_path


def kernel(x, Wq, Wk, Wv, Wo, q_scale, k_scale, v_scale, o_scale):
    B = x.shape[0]
    x = np.asarray(x, dtype=np.float32)
    wq_f = (np.asarray(Wq).T * np.asarray(q_scale).reshape(1, -1)).astype(np.float32)
    wk_f = (np.asarray(Wk).T * np.asarray(k_scale).reshape(1, -1)).astype(np.float32)
    wv_f = (np.asarray(Wv).T * np.asarray(v_scale).reshape(1, -1)).astype(np.float32)
    wo_f = (np.asarray(Wo).T * np.asarray(o_scale).reshape(1, -1)).astype(np.float32)

    nc = _build()
    res = run_bass_kernel_spmd(nc, _in_maps(x, wq_f, wk_f, wv_f, wo_f),
                               core_ids=list(range(8)))
    out = np.zeros((B, S, D), dtype=np.float32)
    for core in range(8):
        b = core // 4
        out[b] += res.results[core]["out_t"].T
    return out



# revision 3
# speedup vs baseline: 1.4019x; 1.4019x over previous
"""Multi-head self-attention (B=2, S=2048, D=1024, H=16) on 8 Trainium2 NeuronCores.

Sharding: batch x head-group. Core c = b*4 + g handles batch b and heads 4g..4g+3
(Megatron-style TP: Wq/Wk/Wv column-sharded, Wo row-sharded; partial outputs
summed on the host).

v2: bf16 matmul operands everywhere (FWL fast weight load), software-pipelined
attention (scores -> exp -> ctx per 128-key chunk, double-buffered score PSUM),
col-tiled ctx matmuls (two heads share one PSUM bank), matmul-computed softmax
denominators (ones-stationary M=1 col tiles), DRAM-bounce partition broadcast +
reciprocal_approx_fast + fused scalar_tensor_tensor normalization.

Per-core layout ("T-layout": sequence on the free dim everywhere):
  inputs: xt [1024, 2048] = x[b].T (bf16); wq/wk/wv [1024, 256] (scale-folded,
  transposed, bf16); wo [256, 1024] (bf16)
  QT/KT = (w.T @ xt) [256, 2048]  d' on partitions, head pair-stacked bf16
  V natural [2048, 256] stored as va2[kpart, hp, kc, 128] (head pair side-by-side)
  per (n, hp, kc): scoresT[k,q] two row-tiled K=64 MMs -> psum [128, 2, 512]
    expT = exp(scoresT/8) -> bf16 (no max subtraction: |scores/8| <~ 2.5)
    ctx  += V_even^T expT_even | V_odd^T expT_odd   (col-tiled, one bank)
    den  += ones^T expT (M=1 col tiles at psum parts 0 / 32, one bank)
  normalize: 1/den broadcast via DRAM bounce, ctxT * bc -> ct (bf16, fused STT)
  outT_partial = wo.T @ ctxT [1024, 2048] (fp32 out)
Host: out[b] = sum_g outT[b, g].T
"""
import sys

sys.path.insert(0, "/opt/trn_rl_repo")

import numpy as np
import ml_dtypes

import concourse.bass as bass
import concourse.tile as tile
from concourse import bacc, mybir
from concourse.bass_utils import run_bass_kernel_spmd

F32 = mybir.dt.float32
BF16 = mybir.dt.bfloat16
NP_BF16 = ml_dtypes.bfloat16

S = 2048          # sequence length per batch
D = 1024          # embedding dim
HG = 4            # heads per core
HD = 64           # head dim
GC = HG * HD      # group cols = 256
P = 128
NQ = 4            # q chunks of 512
QW = 512          # q chunk width
NKC = 16          # key-position chunks of 128
KO = 8            # contraction chunks of 128 over D

_NC_CACHE = {}


def _build():
    if "nc" in _NC_CACHE:
        return _NC_CACHE["nc"]
    nc = bacc.Bacc(trn_type="TRN2", target_bir_lowering=False, debug=False)
    xt_d = nc.dram_tensor("xt", [D, S], BF16, kind="ExternalInput")
    wq_d = nc.dram_tensor("wq", [D, GC], BF16, kind="ExternalInput")
    wk_d = nc.dram_tensor("wk", [D, GC], BF16, kind="ExternalInput")
    wv_d = nc.dram_tensor("wv", [D, GC], BF16, kind="ExternalInput")
    wo_d = nc.dram_tensor("wo", [GC, D], BF16, kind="ExternalInput")
    out_d = nc.dram_tensor("out_t", [D, S], F32, kind="ExternalOutput")
    scr_d = nc.dram_tensor("nrm_scratch", [NQ, 2, 2, QW], F32)
    with tile.TileContext(nc) as tc:
        _emit(nc, tc, xt_d, wq_d, wk_d, wv_d, wo_d, out_d, scr_d)
    nc.compile()
    _NC_CACHE["nc"] = nc
    return nc


def _emit(nc, tc, xt_d, wq_d, wk_d, wv_d, wo_d, out_d, scr_d):
    with tc.tile_pool(name="big", bufs=1) as big, \
         tc.tile_pool(name="expool", bufs=18) as expool, \
         tc.tile_pool(name="evac", bufs=3) as evac, \
         tc.tile_pool(name="nrm", bufs=2) as nrm, \
         tc.tile_pool(name="ps_sp", bufs=2, space="PSUM") as ps_sp, \
         tc.tile_pool(name="ps_ctx", bufs=2, space="PSUM") as ps_ctx, \
         tc.tile_pool(name="ps_den", bufs=1, space="PSUM") as ps_den, \
         tc.tile_pool(name="ps_o", bufs=1, space="PSUM") as ps_o:
        # ---- persistent SBUF tensors ----
        qt = big.tile([P, 2, S], BF16)        # head h at parts (h%2)*64, chunk h//2
        kt = big.tile([P, 2, S], BF16)
        va2 = big.tile([P, 2, NKC, P], BF16)  # [kpart, hp, kc, head-even|head-odd]
        ct = big.tile([P, 2, S], BF16)        # normalized ctxT, same layout as qt
        wo_sb = big.tile([P, 2, D], BF16)
        ones_w = big.tile([P, 1], BF16)       # denominator stationary
        xs = big.tile([P, KO, S], BF16)       # x.T, [d_in(128) x ko x s]
        wq = big.tile([P, KO, GC], BF16)
        wk = big.tile([P, KO, GC], BF16)
        wv = big.tile([P, KO, GC], BF16)

        nc.vector.memset(ones_w[:].bitcast(mybir.dt.uint16), 0x3F80)
        nc.sync.dma_start(wo_sb[:], wo_d.rearrange("(c p) e -> p c e", p=P))
        for ko in range(KO):
            nc.sync.dma_start(xs[:, ko, :], xt_d[ko * P:(ko + 1) * P, :])
        nc.sync.dma_start(wq[:], wq_d.rearrange("(ko p) m -> p ko m", p=P))
        nc.sync.dma_start(wk[:], wk_d.rearrange("(ko p) m -> p ko m", p=P))
        nc.sync.dma_start(wv[:], wv_d.rearrange("(ko p) m -> p ko m", p=P))

        def proj_combo(w_sb, dst, m, n):
            """dst[:, m, n*QW:+QW] = (w_sb chunk).T @ xs chunk, K=128 x 8."""
            pp = ps_sp.tile([P, 2, QW], F32, tag="sp")
            for ko in range(KO):
                nc.tensor.matmul(pp[:, 0, :],
                                 w_sb[:, ko, m * P:(m + 1) * P],
                                 xs[:, ko, n * QW:(n + 1) * QW],
                                 start=(ko == 0), stop=(ko == KO - 1))
            nc.vector.tensor_copy(dst[:, m, n * QW:(n + 1) * QW], pp[:, 0, :])

        def v_combo(sc):
            """va2[:, :, sc, :] = V rows sc*128..+128 (natural layout)."""
            pp = ps_sp.tile([P, 2, QW], F32, tag="sp")
            for ko in range(KO):
                nc.tensor.matmul(pp[:, 0, 0:GC],
                                 xs[:, ko, sc * P:(sc + 1) * P],
                                 wv[:, ko, :],
                                 start=(ko == 0), stop=(ko == KO - 1))
            nc.vector.tensor_copy(
                va2[:, :, sc, :],
                pp[:, 0, 0:GC].rearrange("p (h c) -> p h c", c=P))

        def scores_exp(hp, n, kc):
            """Returns ex tile [P, 2, QW] bf16 = exp(scoresT/8) for both heads."""
            sp = ps_sp.tile([P, 2, QW], F32, tag="sp")
            for e in range(2):
                lo = e * 64
                nc.tensor.matmul(
                    sp[:, e, :],
                    kt[lo:lo + 64, hp, kc * P:(kc + 1) * P],
                    qt[lo:lo + 64, hp, n * QW:(n + 1) * QW],
                    start=True, stop=True)
            ex = expool.tile([P, 2, QW], BF16, tag="ex")
            nc.scalar.activation(
                ex[:].rearrange("p a b -> p (a b)"),
                sp[:].rearrange("p a b -> p (a b)"),
                mybir.ActivationFunctionType.Exp,
                scale=0.125)
            return ex

        def ctx_den(hp, n, kc, ex, ctx_ps, den_ps):
            first, last = kc == 0, kc == NKC - 1
            nc.tensor.matmul(ctx_ps[0:64, :], va2[:, hp, kc, 0:64],
                             ex[:, 0, :], start=first, stop=last,
                             tile_position=(0, 0))
            nc.tensor.matmul(ctx_ps[64:128, :], va2[:, hp, kc, 64:128],
                             ex[:, 1, :], start=first, stop=last,
                             tile_position=(0, 64))
            nc.tensor.matmul(den_ps[0:1, :], ones_w[:, 0:1],
                             ex[:, 0, :], start=first, stop=last,
                             tile_position=(0, 0))
            nc.tensor.matmul(den_ps[32:33, :], ones_w[:, 0:1],
                             ex[:, 1, :], start=first, stop=last,
                             tile_position=(0, 32))

        def normalize(hp, n, ctx_ps, den_ps):
            """ct[:, hp, n] = ctx_ps * (1/den) with den broadcast over parts."""
            ns = slice(n * QW, (n + 1) * QW)
            dsb = nrm.tile([P, QW], F32, tag="dsb")
            for e in range(2):
                nc.vector.tensor_copy(dsb[32 * e:32 * e + 1, :],
                                      den_ps[32 * e:32 * e + 1, :])
            for e in range(2):
                sl = scr_d[n, hp, e]
                nc.sync.dma_start(sl.unsqueeze(0), dsb[32 * e:32 * e + 1, :])
            bc = nrm.tile([P, QW], F32, tag="bc")
            for e in range(2):
                sl = scr_d[n, hp, e]
                bc_src = bass.AP(tensor=sl.tensor, offset=sl.offset,
                                 ap=[[0, 64]] + list(sl.ap))
                nc.sync.dma_start(bc[64 * e:64 * (e + 1), :], bc_src)
            nc.vector.reciprocal_approx_fast(bc[:], bc[:])
            nc.vector.scalar_tensor_tensor(
                ct[:, hp, ns], ctx_ps[:], 1.0, bc[:],
                mybir.AluOpType.mult, mybir.AluOpType.mult)

        def out_proj(n):
            ns = slice(n * QW, (n + 1) * QW)
            for m in range(KO):
                po = ps_o.tile([P, QW], F32, tag="po")
                nc.tensor.matmul(po[:], wo_sb[:, 0, m * P:(m + 1) * P],
                                 ct[:, 0, ns], start=True, stop=False)
                nc.tensor.matmul(po[:], wo_sb[:, 1, m * P:(m + 1) * P],
                                 ct[:, 1, ns], start=False, stop=True)
                ot = evac.tile([P, QW], F32, tag="ot")
                nc.vector.tensor_copy(ot[:], po[:])
                nc.sync.dma_start(out_d[m * P:(m + 1) * P, ns], ot[:])

        # ---- emission order == per-engine execution order; interleave so the
        # scalar engine (exp, the bottleneck) starts early and never starves.
        for m in range(2):                     # KT fully (scores need all kc)
            for n in range(NQ):
                proj_combo(wk, kt, m, n)
        for m in range(2):                     # QT chunk n=0 only
            proj_combo(wq, qt, m, 0)

        # (n0, hp0): scores+exp stream while V is still being projected;
        # ctx/den catch up afterwards (ex tiles stay live in the ring).
        ctx0 = ps_ctx.tile([P, QW], F32, tag="ctx")
        den0 = ps_den.tile([P, QW], F32, tag="den")
        ex_held = [scores_exp(0, 0, kc) for kc in range(NKC)]
        for sc in range(NKC):
            v_combo(sc)

        # (n0, hp1) scores/exp interleaved with (n0, hp0) ctx/den catch-up
        ctx1 = ps_ctx.tile([P, QW], F32, tag="ctx")
        ex1_held = []
        for kc in range(NKC):
            ex1_held.append(scores_exp(1, 0, kc))
            ctx_den(0, 0, kc, ex_held[kc], ctx0, den0)
        normalize(0, 0, ctx0, den0)

        # (n0, hp1) ctx/den + remaining QT chunks
        den1 = ps_den.tile([P, QW], F32, tag="den")
        for kc in range(NKC):
            ctx_den(1, 0, kc, ex1_held[kc], ctx1, den1)
        normalize(1, 0, ctx1, den1)
        for n in range(1, NQ):
            for m in range(2):
                proj_combo(wq, qt, m, n)

        # steady state: per-kc fused pipeline
        for n in range(1, NQ):
            for hp in range(2):
                ctx_ps = ps_ctx.tile([P, QW], F32, tag="ctx")
                den_ps = ps_den.tile([P, QW], F32, tag="den")
                for kc in range(NKC):
                    ex = scores_exp(hp, n, kc)
                    ctx_den(hp, n, kc, ex, ctx_ps, den_ps)
                normalize(hp, n, ctx_ps, den_ps)
            out_proj(n - 1)
        out_proj(NQ - 1)


def _in_maps(x, wq_f, wk_f, wv_f, wo_f):
    maps = []
    for core in range(8):
        b, g = core // 4, core % 4
        cols = slice(g * GC, (g + 1) * GC)
        maps.append({
            "xt": np.ascontiguousarray(x[b].T).astype(NP_BF16),
            "wq": np.ascontiguousarray(wq_f[:, cols]).astype(NP_BF16),
            "wk": np.ascontiguousarray(wk_f[:, cols]).astype(NP_BF16),
            "wv": np.ascontiguousarray(wv_f[:, cols]).astype(NP_BF16),
            "wo": np.ascontiguousarray(wo_f[cols, :]).astype(NP_BF16),
        })
    return maps


def _prep(x, Wq, Wk, Wv, Wo, q_scale, k_scale, v_scale, o_scale):
    x = np.asarray(x, dtype=np.float32)
    wq_f = (np.asarray(Wq).T * np.asarray(q_scale).reshape(1, -1)).astype(np.float32)
    wk_f = (np.asarray(Wk).T * np.asarray(k_scale).reshape(1, -1)).astype(np.float32)
    wv_f = (np.asarray(Wv).T * np.asarray(v_scale).reshape(1, -1)).astype(np.float32)
    wo_f = (np.asarray(Wo).T * np.asarray(o_scale).reshape(1, -1)).astype(np.float32)
    return x, _in_maps(x, wq_f, wk_f, wv_f, wo_f)


def run_traced(x, Wq, Wk, Wv, Wo, q_scale, k_scale, v_scale, o_scale):
    """Like kernel() but with NTFF tracing; returns (out, exec_time_ns, trace_path)."""
    x, maps = _prep(x, Wq, Wk, Wv, Wo, q_scale, k_scale, v_scale, o_scale)
    nc = _build()
    res = run_bass_kernel_spmd(nc, maps, core_ids=list(range(8)), trace=True)
    out = np.zeros((x.shape[0], S, D), dtype=np.float32)
    for core in range(8):
        out[core // 4] += res.results[core]["out_t"].T
    trace_path = None
    if res.instructions_and_trace is not None:
        trace_path = res.instructions_and_trace[1]
    return out, res.exec_time_ns, trace_path


def kernel(x, Wq, Wk, Wv, Wo, q_scale, k_scale, v_scale, o_scale):
    x, maps = _prep(x, Wq, Wk, Wv, Wo, q_scale, k_scale, v_scale, o_scale)
    nc = _build()
    res = run_bass_kernel_spmd(nc, maps, core_ids=list(range(8)))
    out = np.zeros((x.shape[0], S, D), dtype=np.float32)
    for core in range(8):
        out[core // 4] += res.results[core]["out_t"].T
    return out


# revision 5
# speedup vs baseline: 1.7599x; 1.2553x over previous
"""Multi-head self-attention (B=2, S=2048, D=1024, H=16) on 8 Trainium2 NeuronCores.

Sharding: batch x head-group. Core c = b*4 + g handles batch b and heads 4g..4g+3
(Megatron-style TP: Wq/Wk/Wv column-sharded, Wo row-sharded; partial outputs
summed on the host).

v3: bf16 matmuls, phase-pipelined attention. The 8 (n, hp) streams are
processed one per phase; in phase k the PE computes scores for stream k while
ctx / denominator matmuls consume stream k-1's exp tiles (held in a ring), so
the scalar engine (exp: the roofline engine at ~135us) is continuously fed and
the PE stays dense (keeps the HAM clock at 2.4 GHz). Projections and the
output projection are interleaved as per-phase fillers on their own PSUM bank.

Denominators: 4 M=1 ones-stationary col tiles (psum parts 0/32/64/96 = both
heads of both hp streams of one n) share a single 512-cycle PE slot per kc.
Normalization: denom rows -> DRAM bounce -> partition-broadcast [128,512] ->
reciprocal_approx_fast -> fused scalar_tensor_tensor (ctx * 1/den -> bf16).

PSUM: scores ring 2x[128,2,512] (4 banks) + ctx accumulators (2) + denom (1)
+ proj/out shared bank (1) = 8.
"""
import sys

sys.path.insert(0, "/opt/trn_rl_repo")

import numpy as np
import ml_dtypes

import concourse.bass as bass
import concourse.tile as tile
from concourse import bacc, mybir
from concourse.bass_utils import run_bass_kernel_spmd

F32 = mybir.dt.float32
BF16 = mybir.dt.bfloat16
NP_BF16 = ml_dtypes.bfloat16

S = 2048          # sequence length per batch
D = 1024          # embedding dim
HG = 4            # heads per core
HD = 64           # head dim
GC = HG * HD      # group cols = 256
P = 128
NQ = 4            # q chunks of 512
QW = 512          # q chunk width
NKC = 16          # key-position chunks of 128
KO = 8            # contraction chunks of 128 over D

_NC_CACHE = {}


def _build():
    if "nc" in _NC_CACHE:
        return _NC_CACHE["nc"]
    nc = bacc.Bacc(trn_type="TRN2", target_bir_lowering=False, debug=False)
    xt_d = nc.dram_tensor("xt", [D, S], BF16, kind="ExternalInput")
    wq_d = nc.dram_tensor("wq", [D, GC], BF16, kind="ExternalInput")
    wk_d = nc.dram_tensor("wk", [D, GC], BF16, kind="ExternalInput")
    wv_d = nc.dram_tensor("wv", [D, GC], BF16, kind="ExternalInput")
    wo_d = nc.dram_tensor("wo", [GC, D], BF16, kind="ExternalInput")
    out_d = nc.dram_tensor("out_t", [D, S], F32, kind="ExternalOutput")
    scr_d = nc.dram_tensor("nrm_scratch", [NQ, 2, 2, QW], F32)
    with tile.TileContext(nc) as tc:
        _emit(nc, tc, xt_d, wq_d, wk_d, wv_d, wo_d, out_d, scr_d)
    nc.compile()
    _NC_CACHE["nc"] = nc
    return nc


def _emit(nc, tc, xt_d, wq_d, wk_d, wv_d, wo_d, out_d, scr_d):
    with tc.tile_pool(name="big", bufs=1) as big, \
         tc.tile_pool(name="expool", bufs=22) as expool, \
         tc.tile_pool(name="evac", bufs=4) as evac, \
         tc.tile_pool(name="nrm", bufs=4) as nrm, \
         tc.tile_pool(name="ps_sp", bufs=2, space="PSUM") as ps_sp, \
         tc.tile_pool(name="ps_ctx", bufs=2, space="PSUM") as ps_ctx, \
         tc.tile_pool(name="ps_den", bufs=1, space="PSUM") as ps_den, \
         tc.tile_pool(name="ps_o", bufs=1, space="PSUM") as ps_o:
        # ---- persistent SBUF tensors ----
        qt = big.tile([P, 2, S], BF16)        # head h at parts (h%2)*64, chunk h//2
        kt = big.tile([P, 2, S], BF16)
        va2 = big.tile([P, 2, NKC, P], BF16)  # [kpart, hp, kc, head-even|head-odd]
        ct = big.tile([P, 2, S], BF16)        # normalized ctxT, same layout as qt
        wo_sb = big.tile([P, 2, D], BF16)
        ones_w = big.tile([P, 1], BF16)       # denominator stationary
        xs = big.tile([P, KO, S], BF16)       # x.T, [d_in(128) x ko x s]
        wq = big.tile([P, KO, GC], BF16)
        wk = big.tile([P, KO, GC], BF16)
        wv = big.tile([P, KO, GC], BF16)

        nc.vector.memset(ones_w[:].bitcast(mybir.dt.uint16), 0x3F80)
        nc.sync.dma_start(wo_sb[:], wo_d.rearrange("(c p) e -> p c e", p=P))
        for ko in range(KO):
            nc.sync.dma_start(xs[:, ko, :], xt_d[ko * P:(ko + 1) * P, :])
        nc.sync.dma_start(wq[:], wq_d.rearrange("(ko p) m -> p ko m", p=P))
        nc.sync.dma_start(wk[:], wk_d.rearrange("(ko p) m -> p ko m", p=P))
        nc.sync.dma_start(wv[:], wv_d.rearrange("(ko p) m -> p ko m", p=P))

        def proj_combo(w_sb, dst, m, n):
            """dst[:, m, n*QW:+QW] = (w_sb chunk).T @ xs chunk, K=128 x 8."""
            pp = ps_sp.tile([P, 2, QW], F32, tag="sp")
            for ko in range(KO):
                nc.tensor.matmul(pp[:, 0, :],
                                 w_sb[:, ko, m * P:(m + 1) * P],
                                 xs[:, ko, n * QW:(n + 1) * QW],
                                 start=(ko == 0), stop=(ko == KO - 1))
            nc.vector.tensor_copy(dst[:, m, n * QW:(n + 1) * QW], pp[:, 0, :])

        def v_combo(sc):
            """va2[:, :, sc, :] = V rows sc*128..+128 (natural layout)."""
            pp = ps_sp.tile([P, 2, QW], F32, tag="sp")
            for ko in range(KO):
                nc.tensor.matmul(pp[:, 0, 0:GC],
                                 xs[:, ko, sc * P:(sc + 1) * P],
                                 wv[:, ko, :],
                                 start=(ko == 0), stop=(ko == KO - 1))
            nc.vector.tensor_copy(
                va2[:, :, sc, :],
                pp[:, 0, 0:GC].rearrange("p (h c) -> p h c", c=P))

        def scores_exp(hp, n, kc):
            """Returns ex tile [P, 2, QW] bf16 = exp(scoresT/8) for both heads."""
            sp = ps_sp.tile([P, 2, QW], F32, tag="sp")
            for e in range(2):
                lo = e * 64
                nc.tensor.matmul(
                    sp[:, e, :],
                    kt[lo:lo + 64, hp, kc * P:(kc + 1) * P],
                    qt[lo:lo + 64, hp, n * QW:(n + 1) * QW],
                    start=True, stop=True)
            ex = expool.tile([P, 2, QW], BF16, tag="ex")
            nc.scalar.activation(
                ex[:].rearrange("p a b -> p (a b)"),
                sp[:].rearrange("p a b -> p (a b)"),
                mybir.ActivationFunctionType.Exp,
                scale=0.125)
            return ex

        def ctx_pair(hp, kc, ex, ctx_ps):
            first, last = kc == 0, kc == NKC - 1
            nc.tensor.matmul(ctx_ps[0:64, :], va2[:, hp, kc, 0:64],
                             ex[:, 0, :], start=first, stop=last,
                             tile_position=(0, 0))
            nc.tensor.matmul(ctx_ps[64:128, :], va2[:, hp, kc, 64:128],
                             ex[:, 1, :], start=first, stop=last,
                             tile_position=(0, 64))

        def den4(kc, ex0, ex1, den_ps):
            """One slot: denominators for both heads of both hp streams of n."""
            first, last = kc == 0, kc == NKC - 1
            for j, exs in enumerate((ex0[:, 0, :], ex0[:, 1, :],
                                     ex1[:, 0, :], ex1[:, 1, :])):
                c = 32 * j
                nc.tensor.matmul(den_ps[c:c + 1, :], ones_w[:, 0:1], exs,
                                 start=first, stop=last, tile_position=(0, c))

        def normalize(n, hp, ctx_ps, den_ps):
            """ct[:, hp, n] = ctx_ps * (1/den); den rows at parts 64*hp+{0,32}."""
            ns = slice(n * QW, (n + 1) * QW)
            base = 64 * hp
            dsb = nrm.tile([P, QW], F32, tag="dsb")
            for e in range(2):
                r = base + 32 * e
                nc.vector.tensor_copy(dsb[r:r + 1, :], den_ps[r:r + 1, :])
            for e in range(2):
                r = base + 32 * e
                sl = scr_d[n, hp, e]
                nc.sync.dma_start(sl.unsqueeze(0), dsb[r:r + 1, :])
            bc = nrm.tile([P, QW], F32, tag="bc")
            for e in range(2):
                sl = scr_d[n, hp, e]
                bc_src = bass.AP(tensor=sl.tensor, offset=sl.offset,
                                 ap=[[0, 64]] + list(sl.ap))
                nc.sync.dma_start(bc[64 * e:64 * (e + 1), :], bc_src)
            nc.vector.reciprocal_approx_fast(bc[:], bc[:])
            nc.vector.scalar_tensor_tensor(
                ct[:, hp, ns], ctx_ps[:], 1.0, bc[:],
                mybir.AluOpType.mult, mybir.AluOpType.mult)

        def out_chunk(n, m):
            ns = slice(n * QW, (n + 1) * QW)
            po = ps_o.tile([P, QW], F32, tag="po")
            nc.tensor.matmul(po[:], wo_sb[:, 0, m * P:(m + 1) * P],
                             ct[:, 0, ns], start=True, stop=False)
            nc.tensor.matmul(po[:], wo_sb[:, 1, m * P:(m + 1) * P],
                             ct[:, 1, ns], start=False, stop=True)
            ot = evac.tile([P, QW], F32, tag="ot")
            nc.vector.tensor_copy(ot[:], po[:])
            nc.sync.dma_start(out_d[m * P:(m + 1) * P, ns], ot[:])

        # ---- schedule: emission order == per-engine execution order ----
        streams = [(n, hp) for n in range(NQ) for hp in range(2)]

        # P0 (dense warm-up): minimum to unblock stream 0 = (n0, h0)
        proj_combo(wk, kt, 0, 0)
        proj_combo(wq, qt, 0, 0)

        # per-phase fillers (emitted one per kc iteration, leftovers at end)
        fillers = [[] for _ in range(8)]
        # P1: V (ctx(s0) needs it in P2), rest of kt, qt(m1, n0) for s1
        fillers[0] = [lambda sc=sc: v_combo(sc) for sc in range(NKC)]
        fillers[0][1:1] = [lambda: proj_combo(wk, kt, 0, 1)]
        fillers[0][5:5] = [lambda: proj_combo(wk, kt, 0, 2)]
        fillers[0][9:9] = [lambda: proj_combo(wk, kt, 0, 3)]
        fillers[0] += [lambda: proj_combo(wk, kt, 1, 0),
                       lambda: proj_combo(wk, kt, 1, 1),
                       lambda: proj_combo(wk, kt, 1, 2),
                       lambda: proj_combo(wk, kt, 1, 3),
                       lambda: proj_combo(wq, qt, 1, 0)]
        fillers[1] = [lambda: proj_combo(wq, qt, 0, 1),
                      lambda: proj_combo(wq, qt, 1, 1)]
        fillers[2] = [lambda: proj_combo(wq, qt, 0, 2),
                      lambda: proj_combo(wq, qt, 1, 2)]
        fillers[3] = [lambda m=m: out_chunk(0, m) for m in range(KO)]
        fillers[4] = [lambda: proj_combo(wq, qt, 0, 3),
                      lambda: proj_combo(wq, qt, 1, 3)]
        fillers[5] = [lambda m=m: out_chunk(1, m) for m in range(KO)]
        fillers[6] = []
        fillers[7] = [lambda m=m: out_chunk(2, m) for m in range(KO)]

        prev_ex = None            # stream k-1's exp tiles
        prev_ctx = None           # stream k-1's ctx psum accumulator
        prev_den = None           # den accumulator of stream k-1's n
        for k, (n, hp) in enumerate(streams):
            cur_ex = []
            cur_ctx = ps_ctx.tile([P, QW], F32, tag="ctx")
            if hp == 1:
                cur_den = ps_den.tile([P, QW], F32, tag="den")
            else:
                cur_den = prev_den
            fq = list(fillers[k])
            for kc in range(NKC):
                cur_ex.append(scores_exp(hp, n, kc))
                if prev_ex is not None:
                    pn, php = streams[k - 1]
                    ctx_pair(php, kc, prev_ex[kc], prev_ctx)
                    if hp == 1:   # pair den for both hp streams of this n
                        den4(kc, prev_ex[kc], cur_ex[kc], cur_den)
                if fq:
                    fq.pop(0)()
            for f in fq:
                f()
            if prev_ex is not None:
                pn, php = streams[k - 1]
                # ctx(s_{k-1}) complete; its den completes with this phase's
                # den4 (php==0) or completed last phase (php==1).
                normalize(pn, php, prev_ctx, cur_den if php == 0 else prev_den)
            prev_ex, prev_ctx, prev_den = cur_ex, cur_ctx, cur_den

        # tail: ctx + normalize for the final stream, then out-proj n2, n3
        n, hp = streams[-1]
        for kc in range(NKC):
            ctx_pair(hp, kc, prev_ex[kc], prev_ctx)
        normalize(n, hp, prev_ctx, prev_den)
        for m in range(KO):
            out_chunk(3, m)


def _in_maps(x, wq_f, wk_f, wv_f, wo_f):
    maps = []
    for core in range(8):
        b, g = core // 4, core % 4
        cols = slice(g * GC, (g + 1) * GC)
        maps.append({
            "xt": np.ascontiguousarray(x[b].T).astype(NP_BF16),
            "wq": np.ascontiguousarray(wq_f[:, cols]).astype(NP_BF16),
            "wk": np.ascontiguousarray(wk_f[:, cols]).astype(NP_BF16),
            "wv": np.ascontiguousarray(wv_f[:, cols]).astype(NP_BF16),
            "wo": np.ascontiguousarray(wo_f[cols, :]).astype(NP_BF16),
        })
    return maps


def _prep(x, Wq, Wk, Wv, Wo, q_scale, k_scale, v_scale, o_scale):
    x = np.asarray(x, dtype=np.float32)
    wq_f = (np.asarray(Wq).T * np.asarray(q_scale).reshape(1, -1)).astype(np.float32)
    wk_f = (np.asarray(Wk).T * np.asarray(k_scale).reshape(1, -1)).astype(np.float32)
    wv_f = (np.asarray(Wv).T * np.asarray(v_scale).reshape(1, -1)).astype(np.float32)
    wo_f = (np.asarray(Wo).T * np.asarray(o_scale).reshape(1, -1)).astype(np.float32)
    return x, _in_maps(x, wq_f, wk_f, wv_f, wo_f)


def run_traced(x, Wq, Wk, Wv, Wo, q_scale, k_scale, v_scale, o_scale):
    """Like kernel() but with NTFF tracing; returns (out, exec_time_ns, trace_path)."""
    x, maps = _prep(x, Wq, Wk, Wv, Wo, q_scale, k_scale, v_scale, o_scale)
    nc = _build()
    res = run_bass_kernel_spmd(nc, maps, core_ids=list(range(8)), trace=True)
    out = np.zeros((x.shape[0], S, D), dtype=np.float32)
    for core in range(8):
        out[core // 4] += res.results[core]["out_t"].T
    trace_path = None
    if res.instructions_and_trace is not None:
        trace_path = res.instructions_and_trace[1]
    return out, res.exec_time_ns, trace_path


def kernel(x, Wq, Wk, Wv, Wo, q_scale, k_scale, v_scale, o_scale):
    x, maps = _prep(x, Wq, Wk, Wv, Wo, q_scale, k_scale, v_scale, o_scale)
    nc = _build()
    res = run_bass_kernel_spmd(nc, maps, core_ids=list(range(8)))
    out = np.zeros((x.shape[0], S, D), dtype=np.float32)
    for core in range(8):
        out[core // 4] += res.results[core]["out_t"].T
    return out


# revision 9
# speedup vs baseline: 1.8471x; 1.0496x over previous
"""Multi-head self-attention (B=2, S=2048, D=1024, H=16) on 8 Trainium2 NeuronCores.

Sharding: batch x head-group. Core c = b*4 + g handles batch b and heads 4g..4g+3
(Megatron-style TP: Wq/Wk/Wv column-sharded, Wo row-sharded; partial outputs
summed on the host).

v3: bf16 matmuls, phase-pipelined attention. The 8 (n, hp) streams are
processed one per phase; in phase k the PE computes scores for stream k while
ctx / denominator matmuls consume stream k-1's exp tiles (held in a ring), so
the scalar engine (exp: the roofline engine at ~135us) is continuously fed and
the PE stays dense (keeps the HAM clock at 2.4 GHz). Projections and the
output projection are interleaved as per-phase fillers on their own PSUM bank.

Denominators: 4 M=1 ones-stationary col tiles (psum parts 0/32/64/96 = both
heads of both hp streams of one n) share a single 512-cycle PE slot per kc.
Normalization: denom rows -> DRAM bounce -> partition-broadcast [128,512] ->
reciprocal_approx_fast -> fused scalar_tensor_tensor (ctx * 1/den -> bf16).

PSUM: scores ring 2x[128,2,512] (4 banks) + ctx accumulators (2) + denom (1)
+ proj/out shared bank (1) = 8.
"""
import sys

sys.path.insert(0, "/opt/trn_rl_repo")

import numpy as np
import ml_dtypes

import concourse.bass as bass
import concourse.tile as tile
from concourse import bacc, mybir
from concourse.bass_utils import run_bass_kernel_spmd

F32 = mybir.dt.float32
BF16 = mybir.dt.bfloat16
NP_BF16 = ml_dtypes.bfloat16

S = 2048          # sequence length per batch
D = 1024          # embedding dim
HG = 4            # heads per core
HD = 64           # head dim
GC = HG * HD      # group cols = 256
P = 128
NQ = 4            # q chunks of 512
QW = 512          # q chunk width
NKC = 16          # key-position chunks of 128
KO = 8            # contraction chunks of 128 over D

_NC_CACHE = {}


def _build():
    if "nc" in _NC_CACHE:
        return _NC_CACHE["nc"]
    nc = bacc.Bacc(trn_type="TRN2", target_bir_lowering=False, debug=False)
    xt_d = nc.dram_tensor("xt", [D, S], BF16, kind="ExternalInput")
    wq_d = nc.dram_tensor("wq", [D, GC], BF16, kind="ExternalInput")
    wk_d = nc.dram_tensor("wk", [D, GC], BF16, kind="ExternalInput")
    wv_d = nc.dram_tensor("wv", [D, GC], BF16, kind="ExternalInput")
    wo_d = nc.dram_tensor("wo", [GC, D], BF16, kind="ExternalInput")
    out_d = nc.dram_tensor("out_t", [D, S], F32, kind="ExternalOutput")
    scr_d = nc.dram_tensor("nrm_scratch", [NQ, 2, 2, QW], F32)
    with tile.TileContext(nc) as tc:
        _emit(nc, tc, xt_d, wq_d, wk_d, wv_d, wo_d, out_d, scr_d)
    nc.compile()
    _NC_CACHE["nc"] = nc
    return nc


def _emit(nc, tc, xt_d, wq_d, wk_d, wv_d, wo_d, out_d, scr_d):
    with tc.tile_pool(name="big", bufs=1) as big, \
         tc.tile_pool(name="expool", bufs=22) as expool, \
         tc.tile_pool(name="evac", bufs=4) as evac, \
         tc.tile_pool(name="nrm", bufs=4) as nrm, \
         tc.tile_pool(name="ps_sp", bufs=2, space="PSUM") as ps_sp, \
         tc.tile_pool(name="ps_ctx", bufs=2, space="PSUM") as ps_ctx, \
         tc.tile_pool(name="ps_den", bufs=1, space="PSUM") as ps_den, \
         tc.tile_pool(name="ps_o", bufs=1, space="PSUM") as ps_o:
        # ---- persistent SBUF tensors ----
        qt = big.tile([P, 2, S], BF16)        # head h at parts (h%2)*64, chunk h//2
        kt = big.tile([P, 2, S], BF16)
        va2 = big.tile([P, 2, NKC, P], BF16)  # [kpart, hp, kc, head-even|head-odd]
        ct = big.tile([P, 2, S], BF16)        # normalized ctxT, same layout as qt
        wo_sb = big.tile([P, 2, D], BF16)
        ones_w = big.tile([P, 1], BF16)       # denominator stationary
        xs = big.tile([P, KO, S], BF16)       # x.T, [d_in(128) x ko x s]
        wq = big.tile([P, KO, GC], BF16)
        wk = big.tile([P, KO, GC], BF16)
        wv = big.tile([P, KO, GC], BF16)

        nc.vector.memset(ones_w[:].bitcast(mybir.dt.uint16), 0x3F80)
        # DMA order drives time-to-first-exp: wk/wq first, then the n0 slice
        # of x, then wv (P1's V combos), the rest of x, wo last. Small pieces
        # spread across the 16 DMA queues (~22 GB/s each).
        for h in range(2):
            ks = slice(h * 4, h * 4 + 4)
            nc.sync.dma_start(
                wk[:, ks, :],
                wk_d[h * D // 2:(h + 1) * D // 2, :].rearrange(
                    "(ko p) m -> p ko m", p=P))
            nc.sync.dma_start(
                wq[:, ks, :],
                wq_d[h * D // 2:(h + 1) * D // 2, :].rearrange(
                    "(ko p) m -> p ko m", p=P))
        for ko in range(KO):
            nc.sync.dma_start(xs[:, ko, 0:QW], xt_d[ko * P:(ko + 1) * P, 0:QW])
        for h in range(2):
            ks = slice(h * 4, h * 4 + 4)
            nc.sync.dma_start(
                wv[:, ks, :],
                wv_d[h * D // 2:(h + 1) * D // 2, :].rearrange(
                    "(ko p) m -> p ko m", p=P))
        for n in range(1, NQ):
            for ko in range(KO):
                nc.sync.dma_start(xs[:, ko, n * QW:(n + 1) * QW],
                                  xt_d[ko * P:(ko + 1) * P, n * QW:(n + 1) * QW])
        for c in range(4):
            nc.sync.dma_start(
                wo_sb[:, c // 2, (c % 2) * QW:(c % 2 + 1) * QW],
                wo_d[(c // 2) * P:(c // 2 + 1) * P,
                     (c % 2) * QW:(c % 2 + 1) * QW])

        def proj_combo(w_sb, dst, m, n):
            """dst[:, m, n*QW:+QW] = (w_sb chunk).T @ xs chunk, K=128 x 8."""
            pp = ps_sp.tile([P, 2, QW], F32, tag="sp")
            for ko in range(KO):
                nc.tensor.matmul(pp[:, 0, :],
                                 w_sb[:, ko, m * P:(m + 1) * P],
                                 xs[:, ko, n * QW:(n + 1) * QW],
                                 start=(ko == 0), stop=(ko == KO - 1))
            nc.vector.tensor_copy(dst[:, m, n * QW:(n + 1) * QW], pp[:, 0, :])

        def v_combo(sc):
            """va2[:, :, sc, :] = V rows sc*128..+128 (natural layout)."""
            pp = ps_sp.tile([P, 2, QW], F32, tag="sp")
            for ko in range(KO):
                nc.tensor.matmul(pp[:, 0, 0:GC],
                                 xs[:, ko, sc * P:(sc + 1) * P],
                                 wv[:, ko, :],
                                 start=(ko == 0), stop=(ko == KO - 1))
            nc.vector.tensor_copy(
                va2[:, :, sc, :],
                pp[:, 0, 0:GC].rearrange("p (h c) -> p h c", c=P))

        def scores_exp(hp, n, kc):
            """Returns ex tile [P, 2, QW] bf16 = exp(scoresT/8) for both heads."""
            sp = ps_sp.tile([P, 2, QW], F32, tag="sp")
            for e in range(2):
                lo = e * 64
                nc.tensor.matmul(
                    sp[:, e, :],
                    kt[lo:lo + 64, hp, kc * P:(kc + 1) * P],
                    qt[lo:lo + 64, hp, n * QW:(n + 1) * QW],
                    start=True, stop=True)
            ex = expool.tile([P, 2, QW], BF16, tag="ex")
            nc.scalar.activation(
                ex[:].rearrange("p a b -> p (a b)"),
                sp[:].rearrange("p a b -> p (a b)"),
                mybir.ActivationFunctionType.Exp,
                scale=0.125)
            return ex

        def ctx_pair(hp, kc, ex, ctx_ps):
            first, last = kc == 0, kc == NKC - 1
            nc.tensor.matmul(ctx_ps[0:64, :], va2[:, hp, kc, 0:64],
                             ex[:, 0, :], start=first, stop=last,
                             tile_position=(0, 0))
            nc.tensor.matmul(ctx_ps[64:128, :], va2[:, hp, kc, 64:128],
                             ex[:, 1, :], start=first, stop=last,
                             tile_position=(0, 64))

        def den4(kc, ex0, ex1, den_ps):
            """One slot: denominators for both heads of both hp streams of n."""
            first, last = kc == 0, kc == NKC - 1
            for j, exs in enumerate((ex0[:, 0, :], ex0[:, 1, :],
                                     ex1[:, 0, :], ex1[:, 1, :])):
                c = 32 * j
                nc.tensor.matmul(den_ps[c:c + 1, :], ones_w[:, 0:1], exs,
                                 start=first, stop=last, tile_position=(0, c))

        def den_chain(n, hp, den_ps):
            """1/den broadcast tile for (n, hp): DRAM bounce + fast recip."""
            base = 64 * hp
            dsb = nrm.tile([P, QW], F32, tag="dsb")
            for e in range(2):
                r = base + 32 * e
                nc.vector.tensor_copy(dsb[r:r + 1, :], den_ps[r:r + 1, :])
            for e in range(2):
                r = base + 32 * e
                sl = scr_d[n, hp, e]
                nc.sync.dma_start(sl.unsqueeze(0), dsb[r:r + 1, :])
            bc = nrm.tile([P, QW], F32, tag="bc")
            for e in range(2):
                sl = scr_d[n, hp, e]
                bc_src = bass.AP(tensor=sl.tensor, offset=sl.offset,
                                 ap=[[0, 64]] + list(sl.ap))
                nc.sync.dma_start(bc[64 * e:64 * (e + 1), :], bc_src)
            nc.vector.reciprocal_approx_fast(bc[:], bc[:])
            return bc

        def norm_apply(n, hp, ctx_ps, bc):
            ns = slice(n * QW, (n + 1) * QW)
            nc.vector.scalar_tensor_tensor(
                ct[:, hp, ns], ctx_ps[:], 1.0, bc[:],
                mybir.AluOpType.mult, mybir.AluOpType.mult)

        def normalize(n, hp, ctx_ps, den_ps):
            norm_apply(n, hp, ctx_ps, den_chain(n, hp, den_ps))

        def out_chunk(n, m, pool_sp=False):
            ns = slice(n * QW, (n + 1) * QW)
            if pool_sp:
                pot = ps_sp.tile([P, 2, QW], F32, tag="sp", name=f"po_sp_{n}_{m}")
                po = pot[:, 0, :]
            else:
                pot = ps_o.tile([P, QW], F32, tag="po", name=f"po_{n}_{m}")
                po = pot[:]
            nc.tensor.matmul(po, wo_sb[:, 0, m * P:(m + 1) * P],
                             ct[:, 0, ns], start=True, stop=False)
            nc.tensor.matmul(po, wo_sb[:, 1, m * P:(m + 1) * P],
                             ct[:, 1, ns], start=False, stop=True)
            ot = evac.tile([P, QW], F32, tag="ot")
            nc.vector.tensor_copy(ot[:], po)
            nc.sync.dma_start(out_d[m * P:(m + 1) * P, ns], ot[:])

        def proj_halves(w_sb, dst, m, n):
            """proj_combo split in two filler units (smaller PE injections)."""
            cell = {}

            def f1():
                cell["pp"] = ps_sp.tile([P, 2, QW], F32, tag="sp", name=f"ph_{id(cell)}")
                for ko in range(4):
                    nc.tensor.matmul(cell["pp"][:, 0, :],
                                     w_sb[:, ko, m * P:(m + 1) * P],
                                     xs[:, ko, n * QW:(n + 1) * QW],
                                     start=(ko == 0), stop=False)

            def f2():
                pp = cell["pp"]
                for ko in range(4, KO):
                    nc.tensor.matmul(pp[:, 0, :],
                                     w_sb[:, ko, m * P:(m + 1) * P],
                                     xs[:, ko, n * QW:(n + 1) * QW],
                                     start=False, stop=(ko == KO - 1))
                nc.vector.tensor_copy(dst[:, m, n * QW:(n + 1) * QW],
                                      pp[:, 0, :])

            return [f1, f2]

        # ---- schedule: emission order == per-engine execution order ----
        streams = [(n, hp) for n in range(NQ) for hp in range(2)]

        # P0 (dense warm-up): minimum to unblock stream 0 = (n0, h0)
        proj_combo(wk, kt, 0, 0)
        proj_combo(wq, qt, 0, 0)

        # per-phase fillers; pops-per-kc in fillers_rate
        fillers = [[] for _ in range(8)]
        # P1: V (ctx(s0) needs it in P2), rest of kt, qt(m1, n0) for s1.
        # KT m1 / QT m1n0 are needed at P2 kc0 -> interleave them early.
        p1 = []
        extras = [lambda: proj_combo(wk, kt, 0, 1),
                  lambda: proj_combo(wk, kt, 1, 0),
                  lambda: proj_combo(wk, kt, 0, 2),
                  lambda: proj_combo(wk, kt, 1, 1),
                  lambda: proj_combo(wk, kt, 0, 3),
                  lambda: proj_combo(wk, kt, 1, 2),
                  lambda: proj_combo(wk, kt, 1, 3),
                  lambda: proj_combo(wq, qt, 1, 0)]
        for sc in range(NKC):
            p1.append(lambda sc=sc: v_combo(sc))
            if sc < len(extras):
                p1.append(extras[sc])
        fillers[0] = p1
        fillers[1] = proj_halves(wq, qt, 0, 1) + proj_halves(wq, qt, 1, 1)
        fillers[2] = proj_halves(wq, qt, 0, 2) + proj_halves(wq, qt, 1, 2)
        fillers[3] = [lambda m=m: out_chunk(0, m) for m in range(KO)]
        fillers[4] = proj_halves(wq, qt, 0, 3) + proj_halves(wq, qt, 1, 3)
        fillers[5] = [lambda m=m: out_chunk(1, m) for m in range(KO)]
        fillers[6] = []
        fillers[7] = [lambda m=m: out_chunk(2, m) for m in range(KO)]
        rate = [2, 1, 1, 1, 1, 1, 1, 1]

        prev_ex = None            # stream k-1's exp tiles
        prev_ctx = None           # stream k-1's ctx psum accumulator
        prev_den = None           # den accumulator of stream k-1's n
        for k, (n, hp) in enumerate(streams):
            cur_ex = []
            cur_ctx = ps_ctx.tile([P, QW], F32, tag="ctx")
            if hp == 1:
                cur_den = ps_den.tile([P, QW], F32, tag="den")
            else:
                cur_den = prev_den
            fq = list(fillers[k])
            for kc in range(NKC):
                cur_ex.append(scores_exp(hp, n, kc))
                if prev_ex is not None:
                    pn, php = streams[k - 1]
                    ctx_pair(php, kc, prev_ex[kc], prev_ctx)
                    if hp == 1:   # pair den for both hp streams of this n
                        den4(kc, prev_ex[kc], cur_ex[kc], cur_den)
                for _ in range(rate[k]):
                    if fq:
                        fq.pop(0)()
            for f in fq:
                f()
            if prev_ex is not None:
                pn, php = streams[k - 1]
                # ctx(s_{k-1}) complete; its den completes with this phase's
                # den4 (php==0) or completed last phase (php==1).
                normalize(pn, php, prev_ctx, cur_den if php == 0 else prev_den)
            prev_ex, prev_ctx, prev_den = cur_ex, cur_ctx, cur_den

        # tail: den chains prefetch during the final ctx loop, then apply +
        # out-proj(n3) on the freed scores ring (overlapped evacuations).
        n, hp = streams[-1]
        bc_last = den_chain(n, hp, prev_den)
        for kc in range(NKC):
            ctx_pair(hp, kc, prev_ex[kc], prev_ctx)
        norm_apply(n, hp, prev_ctx, bc_last)
        for m in range(KO):
            out_chunk(3, m, pool_sp=True)


def _in_maps(x, wq_f, wk_f, wv_f, wo_f):
    maps = []
    for core in range(8):
        b, g = core // 4, core % 4
        cols = slice(g * GC, (g + 1) * GC)
        maps.append({
            "xt": np.ascontiguousarray(x[b].T).astype(NP_BF16),
            "wq": np.ascontiguousarray(wq_f[:, cols]).astype(NP_BF16),
            "wk": np.ascontiguousarray(wk_f[:, cols]).astype(NP_BF16),
            "wv": np.ascontiguousarray(wv_f[:, cols]).astype(NP_BF16),
            "wo": np.ascontiguousarray(wo_f[cols, :]).astype(NP_BF16),
        })
    return maps


def _prep(x, Wq, Wk, Wv, Wo, q_scale, k_scale, v_scale, o_scale):
    x = np.asarray(x, dtype=np.float32)
    wq_f = (np.asarray(Wq).T * np.asarray(q_scale).reshape(1, -1)).astype(np.float32)
    wk_f = (np.asarray(Wk).T * np.asarray(k_scale).reshape(1, -1)).astype(np.float32)
    wv_f = (np.asarray(Wv).T * np.asarray(v_scale).reshape(1, -1)).astype(np.float32)
    wo_f = (np.asarray(Wo).T * np.asarray(o_scale).reshape(1, -1)).astype(np.float32)
    return x, _in_maps(x, wq_f, wk_f, wv_f, wo_f)


def run_traced(x, Wq, Wk, Wv, Wo, q_scale, k_scale, v_scale, o_scale):
    """Like kernel() but with NTFF tracing; returns (out, exec_time_ns, trace_path)."""
    x, maps = _prep(x, Wq, Wk, Wv, Wo, q_scale, k_scale, v_scale, o_scale)
    nc = _build()
    res = run_bass_kernel_spmd(nc, maps, core_ids=list(range(8)), trace=True)
    out = np.zeros((x.shape[0], S, D), dtype=np.float32)
    for core in range(8):
        out[core // 4] += res.results[core]["out_t"].T
    trace_path = None
    if res.instructions_and_trace is not None:
        trace_path = res.instructions_and_trace[1]
    return out, res.exec_time_ns, trace_path


def kernel(x, Wq, Wk, Wv, Wo, q_scale, k_scale, v_scale, o_scale):
    x, maps = _prep(x, Wq, Wk, Wv, Wo, q_scale, k_scale, v_scale, o_scale)
    nc = _build()
    res = run_bass_kernel_spmd(nc, maps, core_ids=list(range(8)))
    out = np.zeros((x.shape[0], S, D), dtype=np.float32)
    for core in range(8):
        out[core // 4] += res.results[core]["out_t"].T
    return out


# revision 12
# speedup vs baseline: 1.8525x; 1.0029x over previous
"""Multi-head self-attention (B=2, S=2048, D=1024, H=16) on 8 Trainium2 NeuronCores.

Sharding: batch x head-group. Core c = b*4 + g handles batch b and heads 4g..4g+3
(Megatron-style TP: Wq/Wk/Wv column-sharded, Wo row-sharded; partial outputs
summed on the host).

v3: bf16 matmuls, phase-pipelined attention. The 8 (n, hp) streams are
processed one per phase; in phase k the PE computes scores for stream k while
ctx / denominator matmuls consume stream k-1's exp tiles (held in a ring), so
the scalar engine (exp: the roofline engine at ~135us) is continuously fed and
the PE stays dense (keeps the HAM clock at 2.4 GHz). Projections and the
output projection are interleaved as per-phase fillers on their own PSUM bank.

Denominators: 4 M=1 ones-stationary col tiles (psum parts 0/32/64/96 = both
heads of both hp streams of one n) share a single 512-cycle PE slot per kc.
Normalization: denom rows -> DRAM bounce -> partition-broadcast [128,512] ->
reciprocal_approx_fast -> fused scalar_tensor_tensor (ctx * 1/den -> bf16).

PSUM: scores ring 2x[128,2,512] (4 banks) + ctx accumulators (2) + denom (1)
+ proj/out shared bank (1) = 8.
"""
import sys

sys.path.insert(0, "/opt/trn_rl_repo")

import numpy as np
import ml_dtypes

import concourse.bass as bass
import concourse.tile as tile
from concourse import bacc, mybir
from concourse.bass_utils import run_bass_kernel_spmd

F32 = mybir.dt.float32
BF16 = mybir.dt.bfloat16
NP_BF16 = ml_dtypes.bfloat16

S = 2048          # sequence length per batch
D = 1024          # embedding dim
HG = 4            # heads per core
HD = 64           # head dim
GC = HG * HD      # group cols = 256
P = 128
NQ = 4            # q chunks of 512
QW = 512          # q chunk width
NKC = 16          # key-position chunks of 128
KO = 8            # contraction chunks of 128 over D

_NC_CACHE = {}


def _build():
    if "nc" in _NC_CACHE:
        return _NC_CACHE["nc"]
    nc = bacc.Bacc(trn_type="TRN2", target_bir_lowering=False, debug=False)
    xt_d = nc.dram_tensor("xt", [D, S], BF16, kind="ExternalInput")
    wq_d = nc.dram_tensor("wq", [D, GC], BF16, kind="ExternalInput")
    wk_d = nc.dram_tensor("wk", [D, GC], BF16, kind="ExternalInput")
    wv_d = nc.dram_tensor("wv", [D, GC], BF16, kind="ExternalInput")
    wo_d = nc.dram_tensor("wo", [GC, D], BF16, kind="ExternalInput")
    out_d = nc.dram_tensor("out_t", [D, S], F32, kind="ExternalOutput")
    scr_d = nc.dram_tensor("nrm_scratch", [NQ, 2, 2, QW], F32)
    with tile.TileContext(nc) as tc:
        _emit(nc, tc, xt_d, wq_d, wk_d, wv_d, wo_d, out_d, scr_d)
    nc.compile()
    _NC_CACHE["nc"] = nc
    return nc


def _emit(nc, tc, xt_d, wq_d, wk_d, wv_d, wo_d, out_d, scr_d):
    with tc.tile_pool(name="big", bufs=1) as big, \
         tc.tile_pool(name="expool", bufs=22) as expool, \
         tc.tile_pool(name="evac", bufs=4) as evac, \
         tc.tile_pool(name="nrm", bufs=4) as nrm, \
         tc.tile_pool(name="ps_sp", bufs=2, space="PSUM") as ps_sp, \
         tc.tile_pool(name="ps_ctx", bufs=2, space="PSUM") as ps_ctx, \
         tc.tile_pool(name="ps_den", bufs=1, space="PSUM") as ps_den, \
         tc.tile_pool(name="ps_o", bufs=1, space="PSUM") as ps_o:
        # ---- persistent SBUF tensors ----
        qt = big.tile([P, 2, S], BF16)        # head h at parts (h%2)*64, chunk h//2
        kt = big.tile([P, 2, S], BF16)
        va2 = big.tile([P, 2, NKC, P], BF16)  # [kpart, hp, kc, head-even|head-odd]
        ct = big.tile([P, 2, S], BF16)        # normalized ctxT, same layout as qt
        wo_sb = big.tile([P, 2, D], BF16)
        ones_w = big.tile([P, 1], BF16)       # denominator stationary
        xs = big.tile([P, KO, S], BF16)       # x.T, [d_in(128) x ko x s]
        wq = big.tile([P, KO, GC], BF16)
        wk = big.tile([P, KO, GC], BF16)
        wv = big.tile([P, KO, GC], BF16)

        nc.vector.memset(ones_w[:].bitcast(mybir.dt.uint16), 0x3F80)
        # DMA order drives time-to-first-exp: wk/wq first, then the n0 slice
        # of x, then wv (P1's V combos), the rest of x, wo last. Small pieces
        # spread across the 16 DMA queues (~22 GB/s each).
        for h in range(2):
            ks = slice(h * 4, h * 4 + 4)
            nc.sync.dma_start(
                wk[:, ks, :],
                wk_d[h * D // 2:(h + 1) * D // 2, :].rearrange(
                    "(ko p) m -> p ko m", p=P))
        for ko in range(KO):
            nc.sync.dma_start(xs[:, ko, 0:QW], xt_d[ko * P:(ko + 1) * P, 0:QW])
        for h in range(2):
            ks = slice(h * 4, h * 4 + 4)
            nc.sync.dma_start(
                wq[:, ks, :],
                wq_d[h * D // 2:(h + 1) * D // 2, :].rearrange(
                    "(ko p) m -> p ko m", p=P))
        for h in range(2):
            ks = slice(h * 4, h * 4 + 4)
            nc.sync.dma_start(
                wv[:, ks, :],
                wv_d[h * D // 2:(h + 1) * D // 2, :].rearrange(
                    "(ko p) m -> p ko m", p=P))
        for n in range(1, NQ):
            for ko in range(KO):
                nc.sync.dma_start(xs[:, ko, n * QW:(n + 1) * QW],
                                  xt_d[ko * P:(ko + 1) * P, n * QW:(n + 1) * QW])
        for c in range(4):
            nc.sync.dma_start(
                wo_sb[:, c // 2, (c % 2) * QW:(c % 2 + 1) * QW],
                wo_d[(c // 2) * P:(c // 2 + 1) * P,
                     (c % 2) * QW:(c % 2 + 1) * QW])

        def proj_combo(w_sb, dst, m, n):
            """dst[:, m, n*QW:+QW] = (w_sb chunk).T @ xs chunk, K=128 x 8."""
            pp = ps_sp.tile([P, 2, QW], F32, tag="sp")
            for ko in range(KO):
                nc.tensor.matmul(pp[:, 0, :],
                                 w_sb[:, ko, m * P:(m + 1) * P],
                                 xs[:, ko, n * QW:(n + 1) * QW],
                                 start=(ko == 0), stop=(ko == KO - 1))
            nc.vector.tensor_copy(dst[:, m, n * QW:(n + 1) * QW], pp[:, 0, :])

        def v_combo(sc):
            """va2[:, :, sc, :] = V rows sc*128..+128 (natural layout)."""
            pp = ps_sp.tile([P, 2, QW], F32, tag="sp")
            for ko in range(KO):
                nc.tensor.matmul(pp[:, 0, 0:GC],
                                 xs[:, ko, sc * P:(sc + 1) * P],
                                 wv[:, ko, :],
                                 start=(ko == 0), stop=(ko == KO - 1))
            nc.vector.tensor_copy(
                va2[:, :, sc, :],
                pp[:, 0, 0:GC].rearrange("p (h c) -> p h c", c=P))

        def scores_exp(hp, n, kc):
            """Returns ex tile [P, 2, QW] bf16 = exp(scoresT/8) for both heads."""
            sp = ps_sp.tile([P, 2, QW], F32, tag="sp")
            for e in range(2):
                lo = e * 64
                nc.tensor.matmul(
                    sp[:, e, :],
                    kt[lo:lo + 64, hp, kc * P:(kc + 1) * P],
                    qt[lo:lo + 64, hp, n * QW:(n + 1) * QW],
                    start=True, stop=True)
            ex = expool.tile([P, 2, QW], BF16, tag="ex")
            nc.scalar.activation(
                ex[:].rearrange("p a b -> p (a b)"),
                sp[:].rearrange("p a b -> p (a b)"),
                mybir.ActivationFunctionType.Exp,
                scale=0.125)
            return ex

        def ctx_pair(hp, kc, ex, ctx_ps):
            first, last = kc == 0, kc == NKC - 1
            nc.tensor.matmul(ctx_ps[0:64, :], va2[:, hp, kc, 0:64],
                             ex[:, 0, :], start=first, stop=last,
                             tile_position=(0, 0))
            nc.tensor.matmul(ctx_ps[64:128, :], va2[:, hp, kc, 64:128],
                             ex[:, 1, :], start=first, stop=last,
                             tile_position=(0, 64))

        def den4(kc, ex0, ex1, den_ps):
            """One slot: denominators for both heads of both hp streams of n."""
            first, last = kc == 0, kc == NKC - 1
            for j, exs in enumerate((ex0[:, 0, :], ex0[:, 1, :],
                                     ex1[:, 0, :], ex1[:, 1, :])):
                c = 32 * j
                nc.tensor.matmul(den_ps[c:c + 1, :], ones_w[:, 0:1], exs,
                                 start=first, stop=last, tile_position=(0, c))

        def den_chain(n, hp, den_ps):
            """1/den broadcast tile for (n, hp): DRAM bounce + fast recip."""
            base = 64 * hp
            dsb = nrm.tile([P, QW], F32, tag="dsb")
            for e in range(2):
                r = base + 32 * e
                nc.vector.tensor_copy(dsb[r:r + 1, :], den_ps[r:r + 1, :])
            for e in range(2):
                r = base + 32 * e
                sl = scr_d[n, hp, e]
                nc.sync.dma_start(sl.unsqueeze(0), dsb[r:r + 1, :])
            bc = nrm.tile([P, QW], F32, tag="bc")
            for e in range(2):
                sl = scr_d[n, hp, e]
                bc_src = bass.AP(tensor=sl.tensor, offset=sl.offset,
                                 ap=[[0, 64]] + list(sl.ap))
                nc.sync.dma_start(bc[64 * e:64 * (e + 1), :], bc_src)
            nc.vector.reciprocal_approx_fast(bc[:], bc[:])
            return bc

        def norm_apply(n, hp, ctx_ps, bc):
            ns = slice(n * QW, (n + 1) * QW)
            nc.vector.scalar_tensor_tensor(
                ct[:, hp, ns], ctx_ps[:], 1.0, bc[:],
                mybir.AluOpType.mult, mybir.AluOpType.mult)

        def normalize(n, hp, ctx_ps, den_ps):
            norm_apply(n, hp, ctx_ps, den_chain(n, hp, den_ps))

        def out_chunk(n, m, pool_sp=False):
            ns = slice(n * QW, (n + 1) * QW)
            if pool_sp:
                pot = ps_sp.tile([P, 2, QW], F32, tag="sp", name=f"po_sp_{n}_{m}")
                po = pot[:, 0, :]
            else:
                pot = ps_o.tile([P, QW], F32, tag="po", name=f"po_{n}_{m}")
                po = pot[:]
            nc.tensor.matmul(po, wo_sb[:, 0, m * P:(m + 1) * P],
                             ct[:, 0, ns], start=True, stop=False)
            nc.tensor.matmul(po, wo_sb[:, 1, m * P:(m + 1) * P],
                             ct[:, 1, ns], start=False, stop=True)
            ot = evac.tile([P, QW], F32, tag="ot")
            nc.vector.tensor_copy(ot[:], po)
            nc.sync.dma_start(out_d[m * P:(m + 1) * P, ns], ot[:])

        def proj_halves(w_sb, dst, m, n):
            """proj_combo split in two filler units (smaller PE injections)."""
            cell = {}

            def f1():
                cell["pp"] = ps_sp.tile([P, 2, QW], F32, tag="sp", name=f"ph_{id(cell)}")
                for ko in range(4):
                    nc.tensor.matmul(cell["pp"][:, 0, :],
                                     w_sb[:, ko, m * P:(m + 1) * P],
                                     xs[:, ko, n * QW:(n + 1) * QW],
                                     start=(ko == 0), stop=False)

            def f2():
                pp = cell["pp"]
                for ko in range(4, KO):
                    nc.tensor.matmul(pp[:, 0, :],
                                     w_sb[:, ko, m * P:(m + 1) * P],
                                     xs[:, ko, n * QW:(n + 1) * QW],
                                     start=False, stop=(ko == KO - 1))
                nc.vector.tensor_copy(dst[:, m, n * QW:(n + 1) * QW],
                                      pp[:, 0, :])

            return [f1, f2]

        # ---- schedule: emission order == per-engine execution order ----
        streams = [(n, hp) for n in range(NQ) for hp in range(2)]

        # P0 (dense warm-up): minimum to unblock stream 0 = (n0, h0)
        proj_combo(wk, kt, 0, 0)
        proj_combo(wq, qt, 0, 0)

        # per-phase fillers; pops-per-kc in fillers_rate
        fillers = [[] for _ in range(8)]
        # P1: V (ctx(s0) needs it in P2), rest of kt, qt(m1, n0) for s1.
        # KT m1 / QT m1n0 are needed at P2 kc0 -> interleave them early.
        p1 = []
        extras = [lambda: proj_combo(wk, kt, 0, 1),
                  lambda: proj_combo(wk, kt, 1, 0),
                  lambda: proj_combo(wk, kt, 0, 2),
                  lambda: proj_combo(wk, kt, 1, 1),
                  lambda: proj_combo(wk, kt, 0, 3),
                  lambda: proj_combo(wk, kt, 1, 2),
                  lambda: proj_combo(wk, kt, 1, 3),
                  lambda: proj_combo(wq, qt, 1, 0)]
        for sc in range(NKC):
            p1.append(lambda sc=sc: v_combo(sc))
            if sc < len(extras):
                p1.append(extras[sc])
        fillers[0] = p1
        fillers[1] = proj_halves(wq, qt, 0, 1) + proj_halves(wq, qt, 1, 1)
        fillers[2] = proj_halves(wq, qt, 0, 2) + proj_halves(wq, qt, 1, 2)
        fillers[3] = [lambda m=m: out_chunk(0, m) for m in range(KO)]
        fillers[4] = proj_halves(wq, qt, 0, 3) + proj_halves(wq, qt, 1, 3)
        fillers[5] = [lambda m=m: out_chunk(1, m) for m in range(KO)]
        fillers[6] = []
        fillers[7] = [lambda m=m: out_chunk(2, m) for m in range(KO)]
        rate = [2, 1, 1, 1, 1, 1, 1, 1]

        prev_ex = None            # stream k-1's exp tiles
        prev_ctx = None           # stream k-1's ctx psum accumulator
        prev_den = None           # den accumulator of stream k-1's n
        for k, (n, hp) in enumerate(streams):
            cur_ex = []
            cur_ctx = ps_ctx.tile([P, QW], F32, tag="ctx")
            if hp == 1:
                cur_den = ps_den.tile([P, QW], F32, tag="den")
            else:
                cur_den = prev_den
            fq = list(fillers[k])
            for kc in range(NKC):
                cur_ex.append(scores_exp(hp, n, kc))
                if prev_ex is not None:
                    pn, php = streams[k - 1]
                    ctx_pair(php, kc, prev_ex[kc], prev_ctx)
                    if hp == 1:   # pair den for both hp streams of this n
                        den4(kc, prev_ex[kc], cur_ex[kc], cur_den)
                for _ in range(rate[k]):
                    if fq:
                        fq.pop(0)()
            for f in fq:
                f()
            if prev_ex is not None:
                pn, php = streams[k - 1]
                # ctx(s_{k-1}) complete; its den completes with this phase's
                # den4 (php==0) or completed last phase (php==1).
                normalize(pn, php, prev_ctx, cur_den if php == 0 else prev_den)
            prev_ex, prev_ctx, prev_den = cur_ex, cur_ctx, cur_den

        # tail: den chains prefetch during the final ctx loop, then apply +
        # out-proj(n3) on the freed scores ring (overlapped evacuations).
        n, hp = streams[-1]
        bc_last = den_chain(n, hp, prev_den)
        for kc in range(NKC):
            ctx_pair(hp, kc, prev_ex[kc], prev_ctx)
        norm_apply(n, hp, prev_ctx, bc_last)
        for m in range(KO):
            out_chunk(3, m, pool_sp=True)


def _in_maps(x, wq_f, wk_f, wv_f, wo_f):
    maps = []
    for core in range(8):
        b, g = core // 4, core % 4
        cols = slice(g * GC, (g + 1) * GC)
        maps.append({
            "xt": np.ascontiguousarray(x[b].T).astype(NP_BF16),
            "wq": np.ascontiguousarray(wq_f[:, cols]).astype(NP_BF16),
            "wk": np.ascontiguousarray(wk_f[:, cols]).astype(NP_BF16),
            "wv": np.ascontiguousarray(wv_f[:, cols]).astype(NP_BF16),
            "wo": np.ascontiguousarray(wo_f[cols, :]).astype(NP_BF16),
        })
    return maps


def _prep(x, Wq, Wk, Wv, Wo, q_scale, k_scale, v_scale, o_scale):
    x = np.asarray(x, dtype=np.float32)
    wq_f = (np.asarray(Wq).T * np.asarray(q_scale).reshape(1, -1)).astype(np.float32)
    wk_f = (np.asarray(Wk).T * np.asarray(k_scale).reshape(1, -1)).astype(np.float32)
    wv_f = (np.asarray(Wv).T * np.asarray(v_scale).reshape(1, -1)).astype(np.float32)
    wo_f = (np.asarray(Wo).T * np.asarray(o_scale).reshape(1, -1)).astype(np.float32)
    return x, _in_maps(x, wq_f, wk_f, wv_f, wo_f)


def run_traced(x, Wq, Wk, Wv, Wo, q_scale, k_scale, v_scale, o_scale):
    """Like kernel() but with NTFF tracing; returns (out, exec_time_ns, trace_path)."""
    x, maps = _prep(x, Wq, Wk, Wv, Wo, q_scale, k_scale, v_scale, o_scale)
    nc = _build()
    res = run_bass_kernel_spmd(nc, maps, core_ids=list(range(8)), trace=True)
    out = np.zeros((x.shape[0], S, D), dtype=np.float32)
    for core in range(8):
        out[core // 4] += res.results[core]["out_t"].T
    trace_path = None
    if res.instructions_and_trace is not None:
        trace_path = res.instructions_and_trace[1]
    return out, res.exec_time_ns, trace_path


def kernel(x, Wq, Wk, Wv, Wo, q_scale, k_scale, v_scale, o_scale):
    x, maps = _prep(x, Wq, Wk, Wv, Wo, q_scale, k_scale, v_scale, o_scale)
    nc = _build()
    res = run_bass_kernel_spmd(nc, maps, core_ids=list(range(8)))
    out = np.zeros((x.shape[0], S, D), dtype=np.float32)
    for core in range(8):
        out[core // 4] += res.results[core]["out_t"].T
    return out


# revision 13
# speedup vs baseline: 1.8674x; 1.0080x over previous
"""Multi-head self-attention (B=2, S=2048, D=1024, H=16) on 8 Trainium2 NeuronCores.

Sharding: batch x head-group. Core c = b*4 + g handles batch b and heads 4g..4g+3
(Megatron-style TP: Wq/Wk/Wv column-sharded, Wo row-sharded; partial outputs
summed on the host).

v3: bf16 matmuls, phase-pipelined attention. The 8 (n, hp) streams are
processed one per phase; in phase k the PE computes scores for stream k while
ctx / denominator matmuls consume stream k-1's exp tiles (held in a ring), so
the scalar engine (exp: the roofline engine at ~135us) is continuously fed and
the PE stays dense (keeps the HAM clock at 2.4 GHz). Projections and the
output projection are interleaved as per-phase fillers on their own PSUM bank.

Denominators: 4 M=1 ones-stationary col tiles (psum parts 0/32/64/96 = both
heads of both hp streams of one n) share a single 512-cycle PE slot per kc.
Normalization: denom rows -> DRAM bounce -> partition-broadcast [128,512] ->
reciprocal_approx_fast -> fused scalar_tensor_tensor (ctx * 1/den -> bf16).

PSUM: scores ring 2x[128,2,512] (4 banks) + ctx accumulators (2) + denom (1)
+ proj/out shared bank (1) = 8.
"""
import sys

sys.path.insert(0, "/opt/trn_rl_repo")

import numpy as np
import ml_dtypes

import concourse.bass as bass
import concourse.tile as tile
from concourse import bacc, mybir
from concourse.bass_utils import run_bass_kernel_spmd

F32 = mybir.dt.float32
BF16 = mybir.dt.bfloat16
NP_BF16 = ml_dtypes.bfloat16

S = 2048          # sequence length per batch
D = 1024          # embedding dim
HG = 4            # heads per core
HD = 64           # head dim
GC = HG * HD      # group cols = 256
P = 128
NQ = 4            # q chunks of 512
QW = 512          # q chunk width
NKC = 16          # key-position chunks of 128
KO = 8            # contraction chunks of 128 over D

_NC_CACHE = {}


def _build():
    if "nc" in _NC_CACHE:
        return _NC_CACHE["nc"]
    nc = bacc.Bacc(trn_type="TRN2", target_bir_lowering=False, debug=False)
    xt_d = nc.dram_tensor("xt", [D, S], BF16, kind="ExternalInput")
    wq_d = nc.dram_tensor("wq", [D, GC], BF16, kind="ExternalInput")
    wk_d = nc.dram_tensor("wk", [D, GC], BF16, kind="ExternalInput")
    wv_d = nc.dram_tensor("wv", [D, GC], BF16, kind="ExternalInput")
    wo_d = nc.dram_tensor("wo", [GC, D], BF16, kind="ExternalInput")
    out_d = nc.dram_tensor("out_t", [D, S], F32, kind="ExternalOutput")
    scr_d = nc.dram_tensor("nrm_scratch", [NQ, 2, 2, QW], F32)
    with tile.TileContext(nc) as tc:
        _emit(nc, tc, xt_d, wq_d, wk_d, wv_d, wo_d, out_d, scr_d)
    nc.compile()
    _NC_CACHE["nc"] = nc
    return nc


def _emit(nc, tc, xt_d, wq_d, wk_d, wv_d, wo_d, out_d, scr_d):
    with tc.tile_pool(name="big", bufs=1) as big, \
         tc.tile_pool(name="expool", bufs=22) as expool, \
         tc.tile_pool(name="evac", bufs=4) as evac, \
         tc.tile_pool(name="nrm", bufs=4) as nrm, \
         tc.tile_pool(name="ps_sp", bufs=2, space="PSUM") as ps_sp, \
         tc.tile_pool(name="ps_ctx", bufs=2, space="PSUM") as ps_ctx, \
         tc.tile_pool(name="ps_den", bufs=1, space="PSUM") as ps_den, \
         tc.tile_pool(name="ps_o", bufs=1, space="PSUM") as ps_o:
        # ---- persistent SBUF tensors ----
        qt = big.tile([P, 2, S], BF16)        # head h at parts (h%2)*64, chunk h//2
        kt = big.tile([P, 2, S], BF16)
        va2 = big.tile([P, 2, NKC, P], BF16)  # [kpart, hp, kc, head-even|head-odd]
        ct = big.tile([P, 2, S], BF16)        # normalized ctxT, same layout as qt
        wo_sb = big.tile([P, 2, D], BF16)
        ones_w = big.tile([P, 1], BF16)       # denominator stationary
        xs = big.tile([P, KO, S], BF16)       # x.T, [d_in(128) x ko x s]
        wq = big.tile([P, KO, GC], BF16)
        wk = big.tile([P, KO, GC], BF16)
        wv = big.tile([P, KO, GC], BF16)

        nc.vector.memset(ones_w[:].bitcast(mybir.dt.uint16), 0x3F80)
        # DMA order drives time-to-first-exp: wk/wq first, then the n0 slice
        # of x, then wv (P1's V combos), the rest of x, wo last. Small pieces
        # spread across the 16 DMA queues (~22 GB/s each).
        for h in range(2):
            ks = slice(h * 4, h * 4 + 4)
            nc.sync.dma_start(
                wk[:, ks, :],
                wk_d[h * D // 2:(h + 1) * D // 2, :].rearrange(
                    "(ko p) m -> p ko m", p=P))
        for ko in range(KO):
            nc.sync.dma_start(xs[:, ko, 0:QW], xt_d[ko * P:(ko + 1) * P, 0:QW])
        for h in range(2):
            ks = slice(h * 4, h * 4 + 4)
            nc.sync.dma_start(
                wq[:, ks, :],
                wq_d[h * D // 2:(h + 1) * D // 2, :].rearrange(
                    "(ko p) m -> p ko m", p=P))
        for h in range(2):
            ks = slice(h * 4, h * 4 + 4)
            nc.sync.dma_start(
                wv[:, ks, :],
                wv_d[h * D // 2:(h + 1) * D // 2, :].rearrange(
                    "(ko p) m -> p ko m", p=P))
        for n in range(1, NQ):
            for ko in range(KO):
                nc.sync.dma_start(xs[:, ko, n * QW:(n + 1) * QW],
                                  xt_d[ko * P:(ko + 1) * P, n * QW:(n + 1) * QW])
        for c in range(4):
            nc.sync.dma_start(
                wo_sb[:, c // 2, (c % 2) * QW:(c % 2 + 1) * QW],
                wo_d[(c // 2) * P:(c // 2 + 1) * P,
                     (c % 2) * QW:(c % 2 + 1) * QW])

        def proj_combo(w_sb, dst, m, n):
            """dst[:, m, n*QW:+QW] = (w_sb chunk).T @ xs chunk, K=128 x 8."""
            pp = ps_sp.tile([P, 2, QW], F32, tag="sp")
            for ko in range(KO):
                nc.tensor.matmul(pp[:, 0, :],
                                 w_sb[:, ko, m * P:(m + 1) * P],
                                 xs[:, ko, n * QW:(n + 1) * QW],
                                 start=(ko == 0), stop=(ko == KO - 1))
            nc.vector.tensor_copy(dst[:, m, n * QW:(n + 1) * QW], pp[:, 0, :])

        def v_combo(sc):
            """va2[:, :, sc, :] = V rows sc*128..+128 (natural layout)."""
            pp = ps_sp.tile([P, 2, QW], F32, tag="sp")
            for ko in range(KO):
                nc.tensor.matmul(pp[:, 0, 0:GC],
                                 xs[:, ko, sc * P:(sc + 1) * P],
                                 wv[:, ko, :],
                                 start=(ko == 0), stop=(ko == KO - 1))
            nc.vector.tensor_copy(
                va2[:, :, sc, :],
                pp[:, 0, 0:GC].rearrange("p (h c) -> p h c", c=P))

        def scores_exp(hp, n, kc):
            """Returns ex tile [P, 2, QW] bf16 = exp(scoresT/8) for both heads."""
            sp = ps_sp.tile([P, 2, QW], F32, tag="sp")
            for e in range(2):
                lo = e * 64
                nc.tensor.matmul(
                    sp[:, e, :],
                    kt[lo:lo + 64, hp, kc * P:(kc + 1) * P],
                    qt[lo:lo + 64, hp, n * QW:(n + 1) * QW],
                    start=True, stop=True)
            ex = expool.tile([P, 2, QW], BF16, tag="ex")
            nc.scalar.activation(
                ex[:].rearrange("p a b -> p (a b)"),
                sp[:].rearrange("p a b -> p (a b)"),
                mybir.ActivationFunctionType.Exp,
                scale=0.125)
            return ex

        def ctx_pair(hp, kc, ex, ctx_ps):
            first, last = kc == 0, kc == NKC - 1
            nc.tensor.matmul(ctx_ps[0:64, :], va2[:, hp, kc, 0:64],
                             ex[:, 0, :], start=first, stop=last,
                             tile_position=(0, 0))
            nc.tensor.matmul(ctx_ps[64:128, :], va2[:, hp, kc, 64:128],
                             ex[:, 1, :], start=first, stop=last,
                             tile_position=(0, 64))

        def den4(kc, ex0, ex1, den_ps):
            """One slot: denominators for both heads of both hp streams of n."""
            first, last = kc == 0, kc == NKC - 1
            for j, exs in enumerate((ex0[:, 0, :], ex0[:, 1, :],
                                     ex1[:, 0, :], ex1[:, 1, :])):
                c = 32 * j
                nc.tensor.matmul(den_ps[c:c + 1, :], ones_w[:, 0:1], exs,
                                 start=first, stop=last, tile_position=(0, c))

        def den_chain(n, hp, den_ps):
            """1/den broadcast tile for (n, hp): DRAM bounce + fast recip."""
            base = 64 * hp
            dsb = nrm.tile([P, QW], F32, tag="dsb")
            for e in range(2):
                r = base + 32 * e
                nc.vector.tensor_copy(dsb[r:r + 1, :], den_ps[r:r + 1, :])
            for e in range(2):
                r = base + 32 * e
                sl = scr_d[n, hp, e]
                nc.sync.dma_start(sl.unsqueeze(0), dsb[r:r + 1, :])
            bc = nrm.tile([P, QW], F32, tag="bc")
            for e in range(2):
                sl = scr_d[n, hp, e]
                bc_src = bass.AP(tensor=sl.tensor, offset=sl.offset,
                                 ap=[[0, 64]] + list(sl.ap))
                nc.sync.dma_start(bc[64 * e:64 * (e + 1), :], bc_src)
            nc.vector.reciprocal_approx_fast(bc[:], bc[:])
            return bc

        def norm_apply(n, hp, ctx_ps, bc):
            ns = slice(n * QW, (n + 1) * QW)
            nc.vector.scalar_tensor_tensor(
                ct[:, hp, ns], ctx_ps[:], 1.0, bc[:],
                mybir.AluOpType.mult, mybir.AluOpType.mult)

        def normalize(n, hp, ctx_ps, den_ps):
            norm_apply(n, hp, ctx_ps, den_chain(n, hp, den_ps))

        def out_chunk(n, m, pool_sp=False):
            ns = slice(n * QW, (n + 1) * QW)
            if pool_sp:
                pot = ps_sp.tile([P, 2, QW], F32, tag="sp", name=f"po_sp_{n}_{m}")
                po = pot[:, 0, :]
            else:
                pot = ps_o.tile([P, QW], F32, tag="po", name=f"po_{n}_{m}")
                po = pot[:]
            nc.tensor.matmul(po, wo_sb[:, 0, m * P:(m + 1) * P],
                             ct[:, 0, ns], start=True, stop=False)
            nc.tensor.matmul(po, wo_sb[:, 1, m * P:(m + 1) * P],
                             ct[:, 1, ns], start=False, stop=True)
            ot = evac.tile([P, QW], F32, tag="ot")
            nc.vector.tensor_copy(ot[:], po)
            nc.sync.dma_start(out_d[m * P:(m + 1) * P, ns], ot[:])

        def proj_halves(w_sb, dst, m, n):
            """proj_combo split in two filler units (smaller PE injections)."""
            cell = {}

            def f1():
                cell["pp"] = ps_sp.tile([P, 2, QW], F32, tag="sp", name=f"ph_{id(cell)}")
                for ko in range(4):
                    nc.tensor.matmul(cell["pp"][:, 0, :],
                                     w_sb[:, ko, m * P:(m + 1) * P],
                                     xs[:, ko, n * QW:(n + 1) * QW],
                                     start=(ko == 0), stop=False)

            def f2():
                pp = cell["pp"]
                for ko in range(4, KO):
                    nc.tensor.matmul(pp[:, 0, :],
                                     w_sb[:, ko, m * P:(m + 1) * P],
                                     xs[:, ko, n * QW:(n + 1) * QW],
                                     start=False, stop=(ko == KO - 1))
                nc.vector.tensor_copy(dst[:, m, n * QW:(n + 1) * QW],
                                      pp[:, 0, :])

            return [f1, f2]

        # ---- schedule: emission order == per-engine execution order ----
        streams = [(n, hp) for n in range(NQ) for hp in range(2)]

        # PE clock warm-up: dummy matmuls while the input DMAs run, so the
        # HAM un-throttles the PE (1.2 -> 2.4 GHz) before real work.
        dmy_ps = ps_o.tile([P, QW], F32, tag="po", name="dmy_ps")
        for _ in range(64):
            nc.tensor.matmul(dmy_ps[0:1, 0:1], ones_w[:, 0:1], ones_w[:, 0:1],
                             start=True, stop=True)

        # P0 (dense warm-up): minimum to unblock stream 0 = (n0, h0)
        proj_combo(wk, kt, 0, 0)
        proj_combo(wq, qt, 0, 0)

        # per-phase fillers; pops-per-kc in fillers_rate
        fillers = [[] for _ in range(8)]
        # P1: V (ctx(s0) needs it in P2), rest of kt, qt(m1, n0) for s1.
        # KT m1 / QT m1n0 are needed at P2 kc0 -> interleave them early.
        p1 = []
        extras = [lambda: proj_combo(wk, kt, 0, 1),
                  lambda: proj_combo(wk, kt, 1, 0),
                  lambda: proj_combo(wk, kt, 0, 2),
                  lambda: proj_combo(wk, kt, 1, 1),
                  lambda: proj_combo(wk, kt, 0, 3),
                  lambda: proj_combo(wk, kt, 1, 2),
                  lambda: proj_combo(wk, kt, 1, 3),
                  lambda: proj_combo(wq, qt, 1, 0)]
        for sc in range(NKC):
            p1.append(lambda sc=sc: v_combo(sc))
            if sc < len(extras):
                p1.append(extras[sc])
        fillers[0] = p1
        fillers[1] = proj_halves(wq, qt, 0, 1) + proj_halves(wq, qt, 1, 1)
        fillers[2] = proj_halves(wq, qt, 0, 2) + proj_halves(wq, qt, 1, 2)
        fillers[3] = [lambda m=m: out_chunk(0, m) for m in range(KO)]
        fillers[4] = proj_halves(wq, qt, 0, 3) + proj_halves(wq, qt, 1, 3)
        fillers[5] = [lambda m=m: out_chunk(1, m) for m in range(KO)]
        fillers[6] = []
        fillers[7] = [lambda m=m: out_chunk(2, m) for m in range(KO)]
        rate = [2, 1, 1, 1, 1, 1, 1, 1]

        prev_ex = None            # stream k-1's exp tiles
        prev_ctx = None           # stream k-1's ctx psum accumulator
        prev_den = None           # den accumulator of stream k-1's n
        for k, (n, hp) in enumerate(streams):
            cur_ex = []
            cur_ctx = ps_ctx.tile([P, QW], F32, tag="ctx")
            if hp == 1:
                cur_den = ps_den.tile([P, QW], F32, tag="den")
            else:
                cur_den = prev_den
            fq = list(fillers[k])
            for kc in range(NKC):
                cur_ex.append(scores_exp(hp, n, kc))
                if prev_ex is not None:
                    pn, php = streams[k - 1]
                    ctx_pair(php, kc, prev_ex[kc], prev_ctx)
                    if hp == 1:   # pair den for both hp streams of this n
                        den4(kc, prev_ex[kc], cur_ex[kc], cur_den)
                for _ in range(rate[k]):
                    if fq:
                        fq.pop(0)()
            for f in fq:
                f()
            if prev_ex is not None:
                pn, php = streams[k - 1]
                # ctx(s_{k-1}) complete; its den completes with this phase's
                # den4 (php==0) or completed last phase (php==1).
                normalize(pn, php, prev_ctx, cur_den if php == 0 else prev_den)
            prev_ex, prev_ctx, prev_den = cur_ex, cur_ctx, cur_den

        # tail: den chains prefetch during the final ctx loop, then apply +
        # out-proj(n3) on the freed scores ring (overlapped evacuations).
        n, hp = streams[-1]
        bc_last = den_chain(n, hp, prev_den)
        for kc in range(NKC):
            ctx_pair(hp, kc, prev_ex[kc], prev_ctx)
        norm_apply(n, hp, prev_ctx, bc_last)
        for m in range(KO):
            out_chunk(3, m, pool_sp=True)


def _in_maps(x, wq_f, wk_f, wv_f, wo_f):
    maps = []
    for core in range(8):
        b, g = core // 4, core % 4
        cols = slice(g * GC, (g + 1) * GC)
        maps.append({
            "xt": np.ascontiguousarray(x[b].T).astype(NP_BF16),
            "wq": np.ascontiguousarray(wq_f[:, cols]).astype(NP_BF16),
            "wk": np.ascontiguousarray(wk_f[:, cols]).astype(NP_BF16),
            "wv": np.ascontiguousarray(wv_f[:, cols]).astype(NP_BF16),
            "wo": np.ascontiguousarray(wo_f[cols, :]).astype(NP_BF16),
        })
    return maps


def _prep(x, Wq, Wk, Wv, Wo, q_scale, k_scale, v_scale, o_scale):
    x = np.asarray(x, dtype=np.float32)
    wq_f = (np.asarray(Wq).T * np.asarray(q_scale).reshape(1, -1)).astype(np.float32)
    wk_f = (np.asarray(Wk).T * np.asarray(k_scale).reshape(1, -1)).astype(np.float32)
    wv_f = (np.asarray(Wv).T * np.asarray(v_scale).reshape(1, -1)).astype(np.float32)
    wo_f = (np.asarray(Wo).T * np.asarray(o_scale).reshape(1, -1)).astype(np.float32)
    return x, _in_maps(x, wq_f, wk_f, wv_f, wo_f)


def run_traced(x, Wq, Wk, Wv, Wo, q_scale, k_scale, v_scale, o_scale):
    """Like kernel() but with NTFF tracing; returns (out, exec_time_ns, trace_path)."""
    x, maps = _prep(x, Wq, Wk, Wv, Wo, q_scale, k_scale, v_scale, o_scale)
    nc = _build()
    res = run_bass_kernel_spmd(nc, maps, core_ids=list(range(8)), trace=True)
    out = np.zeros((x.shape[0], S, D), dtype=np.float32)
    for core in range(8):
        out[core // 4] += res.results[core]["out_t"].T
    trace_path = None
    if res.instructions_and_trace is not None:
        trace_path = res.instructions_and_trace[1]
    return out, res.exec_time_ns, trace_path


def kernel(x, Wq, Wk, Wv, Wo, q_scale, k_scale, v_scale, o_scale):
    x, maps = _prep(x, Wq, Wk, Wv, Wo, q_scale, k_scale, v_scale, o_scale)
    nc = _build()
    res = run_bass_kernel_spmd(nc, maps, core_ids=list(range(8)))
    out = np.zeros((x.shape[0], S, D), dtype=np.float32)
    for core in range(8):
        out[core // 4] += res.results[core]["out_t"].T
    return out
